# revision 8
# baseline (speedup 1.0000x reference)
"""ChildSumTreeLSTMCell on 8 Trainium2 NeuronCores.

Strategy: sort edges by destination node on the host (index preprocessing as
part of sharding), partition nodes contiguously across the 8 cores so every
core's segment sums are fully local -- zero collectives.  On each core, edges
are packed into node tiles (<=128 nodes, <=512 edges each, 4 chunks of 128
edge slots).  Segment sums become matmuls against a 0/1 membership matrix M
built on-device from the destination indices; the forget-gate gather
f[dst] * c_src factorizes to f * segment_sum(c_src), removing the second
scatter entirely.  All matmuls run in bf16 (inputs stay f32 in HBM).
"""

import sys

for _p in ("/opt/trn_rl_repo", "/root/.axon_site/_ro/trn_rl_repo"):
    if _p not in sys.path:
        sys.path.append(_p)

import numpy as np
import ml_dtypes

import concourse.bass as bass
import concourse.bacc as bacc
import concourse.mybir as mybir
import concourse.tile as tile
from concourse.vector_clock import ScopedClock, VectorClock
from concourse.bass_utils import run_bass_kernel_spmd

F32 = mybir.dt.float32
BF16 = mybir.dt.bfloat16

E = 500_000
N = 125_000
H = 128
G = 64
NCORES = 8
NPC = N // NCORES          # nodes per core
CHUNK = 128                # edges per chunk
CPT = 4                    # chunks per tile
TILE_E = CHUNK * CPT       # edge slots per tile
bf16_np = ml_dtypes.bfloat16


# --- Tile-exit drain workaround -------------------------------------------
# This neuronxcc build caps sync wait commands at one per instruction; Tile's
# exit drain attaches one wait per live proc to a single Drain.  Split them
# across nops.
def _split_drain_and_barrier(self, tick_clock, wait_clock):
    gc = tick_clock.global_clock
    n = len(gc)
    for i in range(n):
        if gc[i] <= 0:
            continue
        sub = VectorClock([gc[j] if j == i else 0 for j in range(n)])
        nop_inst = self.nc.sync.nop(nofuse=True, hint="drain_split")
        wait_clock.add_sem_waits(nop_inst.ins, ScopedClock({None: sub}))
    self.nc.sync.drain()
    self.nc.all_engine_barrier()
    assert self.sems is not None
    popped = self.nc._tile_sem_poison_stack.pop()
    assert popped is self._sem_poison
    self.nc.clear_and_free_semaphores(list(self.sems.allocated().values()))
    self.nc.all_engine_barrier()


# (kept available but not installed -- Bacc.generate_event_semaphores splits
# multi-wait instructions during finalize)
#tile.TileContext._drain_and_barrier = _split_drain_and_barrier


def build_graph(T):
    """Build the per-core Bass graph for T node tiles."""
    nc = bacc.Bacc()
    dp = nc.declare_dram_parameter
    h4_ext = dp("h4", [T, 128, TILE_E], F32, isOutput=False)
    c4_ext = dp("c4", [T, 128, TILE_E], F32, isOutput=False)
    e4_ext = dp("e4", [T, 128, TILE_E], F32, isOutput=False)
    sdT_ext = dp("sdT", [128, T * TILE_E], F32, isOutput=False)
    oh4_ext = dp("oh4", [4, T * TILE_E], BF16, isOutput=False)
    dstT_ext = dp("dstT", [128, T * CPT], F32, isOutput=False)
    welT_ext = dp("welT", [G, 128], F32, isOutput=False)
    wa_ext = dp("wa", [G, 4], F32, isOutput=False)
    bel_ext = dp("belB", [4, 128], F32, isOutput=False)
    wg_ext = {}
    for x in "fiuo":
        wg_ext[x] = dp("w%sT" % x, [2 * H, 128], F32, isOutput=False)
    bias_ext = {}
    for x in "fiuo":
        bias_ext[x] = (dp("bW%s" % x, [H], F32, isOutput=False),
                       dp("b%s" % x, [H], F32, isOutput=False))
    out_ext = dp("outT", [2 * H, T * 128], F32, isOutput=True)

    with tile.TileContext(nc) as tc:
        cst = tc.alloc_tile_pool(name="cst", bufs=1)
        pin = tc.alloc_tile_pool(name="pin", bufs=3)
        pcv = tc.alloc_tile_pool(name="pcv", bufs=2)
        pnd = tc.alloc_tile_pool(name="pnd", bufs=2)
        pacc = tc.alloc_tile_pool(name="pacc", bufs=2, space="PSUM")
        pmm = tc.alloc_tile_pool(name="pmm", bufs=2, space="PSUM")

        # -- setup: constants ------------------------------------------------
        welT_sb = cst.tile([G, 128], F32)
        nc.sync.dma_start(out=welT_sb[:], in_=welT_ext[:])
        wa_sb = cst.tile([G, 4], F32)
        nc.sync.dma_start(out=wa_sb[:], in_=wa_ext[:])
        belr = cst.tile([4, 128], F32)
        nc.sync.dma_start(out=belr[:], in_=bel_ext[:])
        t2p = pmm.tile([4, 128], F32, tag="mm")
        nc.tensor.matmul(out=t2p[:], lhsT=wa_sb[:], rhs=welT_sb[:],
                         start=True, stop=True)
        wel_b16 = cst.tile([G, 128], BF16)
        nc.vector.tensor_copy(out=wel_b16[:], in_=welT_sb[:])
        t4b = cst.tile([4, 128], BF16)
        nc.vector.tensor_tensor(out=t4b[:], in0=t2p[:],
                                in1=belr[:], op=mybir.AluOpType.add)
        wtcomb = cst.tile([G + 4, 128], BF16)
        nc.sync.dma_start(out=wtcomb[0:G, :], in_=wel_b16[:])
        nc.sync.dma_start(out=wtcomb[G:G + 4, :], in_=t4b[:])

        wg = {}
        for x in "fiuo":
            stg = cst.tile([128, 128], F32, tag="wstg_%s" % x)
            nc.sync.dma_start(out=stg[:], in_=wg_ext[x][0:128, :])
            wa_t = cst.tile([128, 128], BF16, tag="wg_%s_a" % x)
            nc.vector.tensor_copy(out=wa_t[:], in_=stg[:])
            stg2 = cst.tile([128, 128], F32, tag="wstg2_%s" % x)
            nc.sync.dma_start(out=stg2[:], in_=wg_ext[x][128:256, :])
            wb_t = cst.tile([128, 128], BF16, tag="wg_%s_b" % x)
            nc.vector.tensor_copy(out=wb_t[:], in_=stg2[:])
            wg[x] = (wa_t, wb_t)

        bias = {}
        for x in "fiuo":
            b1 = cst.tile([128, 1], F32, tag="b1_%s" % x)
            nc.sync.dma_start(out=b1[:], in_=bias_ext[x][0][:, None])
            b2 = cst.tile([128, 1], F32, tag="b2_%s" % x)
            nc.sync.dma_start(out=b2[:], in_=bias_ext[x][1][:, None])
            bs = cst.tile([128, 1], F32, tag="bs_%s" % x)
            nc.vector.tensor_tensor(out=bs[:], in0=b1[:], in1=b2[:],
                                    op=mybir.AluOpType.add)
            bias[x] = bs

        it32 = cst.tile([128, 128], mybir.dt.int32)
        nc.gpsimd.iota(out=it32[:], pattern=[[1, 128]], base=0,
                       channel_multiplier=0)
        iotaF = cst.tile([128, 128], F32)
        nc.vector.tensor_copy(out=iotaF[:], in_=it32[:])

        dstT = cst.tile([128, T * CPT], F32)
        nc.sync.dma_start(out=dstT[:], in_=dstT_ext[:])

        # -- main loop over node tiles --------------------------------------
        for t in range(T):
            h4 = pin.tile([128, TILE_E], F32, tag="h4")
            nc.sync.dma_start(out=h4[:], in_=h4_ext[t])
            c4 = pin.tile([128, TILE_E], F32, tag="c4")
            nc.sync.dma_start(out=c4[:], in_=c4_ext[t])
            e4 = pin.tile([128, TILE_E], F32, tag="e4")
            nc.sync.dma_start(out=e4[:], in_=e4_ext[t])
            sf4 = pin.tile([G, TILE_E], F32, tag="sf4")
            nc.sync.dma_start(out=sf4[:], in_=sdT_ext[0:G, t * TILE_E:(t + 1) * TILE_E])
            df4 = pin.tile([G, TILE_E], F32, tag="df4")
            nc.sync.dma_start(out=df4[:], in_=sdT_ext[G:128, t * TILE_E:(t + 1) * TILE_E])
            B68 = pcv.tile([G + 4, TILE_E], BF16, tag="B68")
            nc.sync.dma_start(out=B68[G:G + 4, :],
                              in_=oh4_ext[:, t * TILE_E:(t + 1) * TILE_E])
            nc.vector.tensor_tensor(out=B68[0:G, :], in0=sf4[:],
                                    in1=df4[:], op=mybir.AluOpType.mult)
            e4b = pcv.tile([128, TILE_E], BF16, tag="e4b")
            nc.scalar.activation(out=e4b[:], in_=e4[:],
                                 func=mybir.ActivationFunctionType.Copy)
            c4b = pcv.tile([128, TILE_E], BF16, tag="c4b")
            nc.gpsimd.tensor_copy(out=c4b[:], in_=c4[:])
            M4 = pcv.tile([128, TILE_E], BF16, tag="M4")
            hw4 = pcv.tile([128, TILE_E], BF16, tag="hw4")

            hs1 = pacc.tile([128, 128], F32, tag="hs1", space="PSUM")
            hs2 = pacc.tile([128, 128], F32, tag="hs2", space="PSUM")
            cs = pacc.tile([128, 128], F32, tag="cs", space="PSUM")

            for c in range(CPT):
                sl = slice(c * CHUNK, (c + 1) * CHUNK)
                ewp = pmm.tile([128, 128], F32, tag="mm", space="PSUM")
                nc.tensor.matmul(out=ewp[:], lhsT=B68[:, sl], rhs=wtcomb[:],
                                 start=True, stop=True)
                nc.vector.tensor_scalar(
                    out=M4[:, sl], in0=iotaF[:],
                    scalar1=dstT[:, t * CPT + c:t * CPT + c + 1], scalar2=None,
                    op0=mybir.AluOpType.is_equal)
                nc.vector.tensor_tensor(out=hw4[:, sl], in0=h4[:, sl],
                                        in1=ewp[:], op=mybir.AluOpType.mult)
                st = c == 0
                sp = c == CPT - 1
                nc.tensor.matmul(out=hs1[:], lhsT=hw4[:, sl], rhs=M4[:, sl],
                                 start=st, stop=sp)
                nc.tensor.matmul(out=hs2[:], lhsT=e4b[:, sl], rhs=M4[:, sl],
                                 start=st, stop=sp)
                nc.tensor.matmul(out=cs[:], lhsT=c4b[:, sl], rhs=M4[:, sl],
                                 start=st, stop=sp)

            hsa = pnd.tile([128, 128], BF16, tag="hsa")
            nc.vector.tensor_copy(out=hsa[:], in_=hs1[:])
            hsb = pnd.tile([128, 128], BF16, tag="hsb")
            nc.vector.tensor_copy(out=hsb[:], in_=hs2[:])

            gate = {}
            for x, fn in (("f", "Sigmoid"), ("i", "Sigmoid"),
                          ("u", "Tanh"), ("o", "Sigmoid")):
                gp = pmm.tile([128, 128], F32, tag="mm", space="PSUM")
                nc.tensor.matmul(out=gp[:], lhsT=wg[x][0][:], rhs=hsa[:],
                                 start=True, stop=False)
                nc.tensor.matmul(out=gp[:], lhsT=wg[x][1][:], rhs=hsb[:],
                                 start=False, stop=True)
                gs = pnd.tile([128, 128], F32, tag="g_%s" % x)
                nc.scalar.activation(
                    out=gs[:], in_=gp[:],
                    func=getattr(mybir.ActivationFunctionType, fn),
                    bias=bias[x][:])
                gate[x] = gs

            ct = pnd.tile([128, 128], F32, tag="ct")
            nc.vector.tensor_tensor(out=ct[:], in0=gate["f"][:], in1=cs[:],
                                    op=mybir.AluOpType.mult)
            iu = pnd.tile([128, 128], F32, tag="iu")
            nc.vector.tensor_tensor(out=iu[:], in0=gate["i"][:],
                                    in1=gate["u"][:], op=mybir.AluOpType.mult)
            cT = pnd.tile([128, 128], F32, tag="cT")
            nc.vector.tensor_tensor(out=cT[:], in0=iu[:], in1=ct[:],
                                    op=mybir.AluOpType.add)
            th = pnd.tile([128, 128], F32, tag="th")
            nc.scalar.activation(out=th[:], in_=cT[:],
                                 func=mybir.ActivationFunctionType.Tanh)
            hT = pnd.tile([128, 128], F32, tag="hT")
            nc.vector.tensor_tensor(out=hT[:], in0=gate["o"][:], in1=th[:],
                                    op=mybir.AluOpType.mult)
            nc.sync.dma_start(out=out_ext[0:128, t * 128:(t + 1) * 128],
                              in_=hT[:])
            nc.sync.dma_start(out=out_ext[128:256, t * 128:(t + 1) * 128],
                              in_=cT[:])

        for p in (pmm, pacc, pnd, pcv, pin, cst):
            p.release()
    nc.finalize()
    return nc


def plan_tiles(dst_local, npc):
    """Greedy node tiling: <=128 nodes and <=TILE_E edges per tile.
    Returns list of (n0, n1, e0, e1) using sorted-edge offsets."""
    cnt = np.bincount(dst_local, minlength=npc)
    cum = np.concatenate([[0], np.cumsum(cnt)])
    tiles = []
    s = 0
    while s < npc:
        hi = min(s + 128, npc)
        m = int(np.searchsorted(cum, cum[s] + TILE_E, side="right")) - 1
        m = max(s + 1, min(hi, m))
        tiles.append((s, m, int(cum[s]), int(cum[m])))
        s = m
    return tiles


def prep_core(k, h_src, c_src, embed_dst, src_f, dst_f, etype, dst, T=None):
    """Build one core's padded, tiled input arrays."""
    lo = k * NPC
    sel = np.nonzero((dst >= lo) & (dst < lo + NPC))[0]
    dl = (dst[sel] - lo).astype(np.int64)
    order = np.argsort(dl, kind="stable")
    eidx = sel[order]
    dls = dl[order]
    tiles = plan_tiles(dls, NPC)
    Tk = len(tiles)
    if T is None:
        T = Tk
    assert Tk <= T
    ES = T * TILE_E
    # slot -> source edge (or -1)
    src_slot = np.full(ES, -1, dtype=np.int64)
    dst_slot = np.full(ES, -1.0, dtype=np.float32)
    for t, (n0, n1, e0, e1) in enumerate(tiles):
        ne = e1 - e0
        assert ne <= TILE_E and n1 - n0 <= 128
        src_slot[t * TILE_E:t * TILE_E + ne] = eidx[e0:e1]
        dst_slot[t * TILE_E:t * TILE_E + ne] = (dls[e0:e1] - n0).astype(np.float32)
    val = src_slot >= 0
    gi = src_slot[val]

    def pad_rows(a, w):
        out = np.zeros((ES, w), dtype=np.float32)
        out[val] = a[gi]
        return out

    hp = pad_rows(h_src, H).reshape(T, CPT, CHUNK, H).transpose(0, 2, 1, 3)
    hp = np.ascontiguousarray(hp.reshape(T, 128, TILE_E))
    cp = pad_rows(c_src, H).reshape(T, CPT, CHUNK, H).transpose(0, 2, 1, 3)
    cp = np.ascontiguousarray(cp.reshape(T, 128, TILE_E))
    ep = pad_rows(embed_dst, H).reshape(T, CPT, CHUNK, H).transpose(0, 2, 1, 3)
    ep = np.ascontiguousarray(ep.reshape(T, 128, TILE_E))
    sd = np.concatenate([pad_rows(src_f, G), pad_rows(dst_f, G)], axis=1)
    sdT = np.ascontiguousarray(sd.T)
    oh = np.zeros((ES, 4), dtype=np.float32)
    oh[val, etype[gi]] = 1.0
    oh[val, 3] = 1.0
    ohT = np.ascontiguousarray(oh.T).astype(bf16_np)
    dstT = np.ascontiguousarray(dst_slot.reshape(T * CPT, CHUNK).T)
    return {"h4": hp, "c4": cp, "e4": ep, "sdT": sdT, "oh4": ohT,
            "dstT": dstT}, tiles, Tk


_graph_cache = {}
TRACE = False          # set by test.py to capture an NTFF profile
LAST = {}              # last run's BassKernelResults


def _install_axon_hook():
    import types, contextlib, ctypes

    def _make_hook(so_path="/opt/axon/libaxon_pjrt.so"):
        lib = ctypes.CDLL(so_path)
        if not hasattr(lib, "axon_start_nrt_profile"):
            return None
        lib.axon_start_nrt_profile.argtypes = [
            ctypes.POINTER(ctypes.c_int64), ctypes.c_size_t]
        lib.axon_start_nrt_profile.restype = ctypes.c_int64
        lib.axon_stop_nrt_profile.argtypes = [ctypes.c_char_p]
        lib.axon_stop_nrt_profile.restype = ctypes.c_int64

        @contextlib.contextmanager
        def hook(output_dir, device_ids):
            import jax
            jax.devices()
            if device_ids:
                ids = (ctypes.c_int64 * len(device_ids))(*device_ids)
                rc = lib.axon_start_nrt_profile(ids, len(device_ids))
            else:
                rc = lib.axon_start_nrt_profile(None, 0)
            if rc != 0:
                raise RuntimeError("axon_start_nrt_profile rc=%d" % rc)
            try:
                yield
            finally:
                n = lib.axon_stop_nrt_profile(str(output_dir).encode())
                print("profile: %d file(s) written to %s" % (n, output_dir),
                      file=sys.stderr)

        return hook

    hook = _make_hook()
    mod = types.ModuleType("antenv.axon_hooks")
    mod.get_axon_ntff_profile_hook = lambda: hook
    mod.set_axon_ntff_profile_hook = lambda h: None
    sys.modules["antenv.axon_hooks"] = mod


def _belB(b_el):
    out = np.zeros((4, 128), dtype=np.float32)
    out[3] = b_el
    return out


def kernel(**inputs):
    h_src = np.asarray(inputs["h_src"], dtype=np.float32)
    c_src = np.asarray(inputs["c_src"], dtype=np.float32)
    embed_dst = np.asarray(inputs["embed_dst"], dtype=np.float32)
    src_f = np.asarray(inputs["src_node_feat"], dtype=np.float32)
    dst_f = np.asarray(inputs["dst_node_feat"], dtype=np.float32)
    etype = np.asarray(inputs["edge_type_idx"]).astype(np.int64)
    dst = np.asarray(inputs["dst_idx"]).astype(np.int64)

    weights = {
        "welT": np.ascontiguousarray(np.asarray(inputs["W_el"], np.float32).T),
        "wa": np.ascontiguousarray(np.concatenate(
            [np.asarray(inputs["W_eoh"], np.float32),
             np.asarray(inputs["b_eoh"], np.float32)[:, None]], axis=1)),
        "belB": _belB(np.asarray(inputs["b_el"], np.float32)),
    }
    for x, wn, bwn, bn in (("f", "Wf", "bWf", "bf"), ("i", "Wi", "bWi", "bi"),
                           ("u", "Wu", "bWu", "bu"), ("o", "Wo", "bWo", "bo")):
        weights["w%sT" % x] = np.ascontiguousarray(
            np.asarray(inputs[wn], np.float32).T)
        weights["bW%s" % x] = np.asarray(inputs[bwn], np.float32)
        weights["b%s" % x] = np.asarray(inputs[bn], np.float32)

    # first pass: tile counts per core
    planned = []
    for k in range(NCORES):
        lo = k * NPC
        sel = np.nonzero((dst >= lo) & (dst < lo + NPC))[0]
        dl = np.sort((dst[sel] - lo).astype(np.int64))
        planned.append(plan_tiles(dl, NPC))
    T = max(len(p) for p in planned)

    in_maps = []
    tiles_all = []
    for k in range(NCORES):
        m, tiles, _ = prep_core(k, h_src, c_src, embed_dst, src_f, dst_f,
                                etype, dst, T=T)
        m.update(weights)
        in_maps.append(m)
        tiles_all.append(tiles)

    if T not in _graph_cache:
        _graph_cache[T] = build_graph(T)
    nc = _graph_cache[T]

    if TRACE:
        _install_axon_hook()
    res = run_bass_kernel_spmd(nc, in_maps, list(range(NCORES)), trace=TRACE)
    LAST["res"] = res

    out = np.empty((N, 2 * H), dtype=np.float32)
    for k in range(NCORES):
        outT = res.results[k]["outT"]
        cols = np.concatenate(
            [t * 128 + np.arange(n1 - n0)
             for t, (n0, n1, _, _) in enumerate(tiles_all[k])])
        out[k * NPC:(k + 1) * NPC] = np.asarray(outT)[:, cols].T
    return out


# revision 10
# speedup vs baseline: 1.2382x; 1.2382x over previous
"""ChildSumTreeLSTMCell on 8 Trainium2 NeuronCores.

Strategy: sort edges by destination node on the host (index preprocessing as
part of sharding), partition nodes contiguously across the 8 cores so every
core's segment sums are fully local -- zero collectives.  On each core, edges
are packed into node tiles (<=128 nodes, <=512 edges each, 4 chunks of 128
edge slots).  Segment sums become matmuls against a 0/1 membership matrix M
built on-device from the destination indices; the forget-gate gather
f[dst] * c_src factorizes to f * segment_sum(c_src), removing the second
scatter entirely.  All matmuls run in bf16 (inputs stay f32 in HBM).

v2: packed DMA streams (hce = h|c|e in one 768KB transfer, sfdf merged,
single merged [h|c] output), DMA issue spread across sync/gpsimd queues,
one-shot M build via free-dim broadcast, merged PSUM evacuation.
"""

import sys

for _p in ("/opt/trn_rl_repo", "/root/.axon_site/_ro/trn_rl_repo"):
    if _p not in sys.path:
        sys.path.append(_p)

import numpy as np
import ml_dtypes

import concourse.bacc as bacc
import concourse.mybir as mybir
import concourse.tile as tile
from concourse.bass_utils import run_bass_kernel_spmd

F32 = mybir.dt.float32
BF16 = mybir.dt.bfloat16

E = 500_000
N = 125_000
H = 128
G = 64
NCORES = 8
NPC = N // NCORES          # nodes per core
CHUNK = 128                # edges per chunk
CPT = 4                    # chunks per tile
TILE_E = CHUNK * CPT       # edge slots per tile
bf16_np = ml_dtypes.bfloat16

TRACE = False              # set by test.py to capture an NTFF profile
LAST = {}                  # last run's BassKernelResults


def _install_axon_hook():
    import types, contextlib, ctypes

    def _make_hook(so_path="/opt/axon/libaxon_pjrt.so"):
        lib = ctypes.CDLL(so_path)
        if not hasattr(lib, "axon_start_nrt_profile"):
            return None
        lib.axon_start_nrt_profile.argtypes = [
            ctypes.POINTER(ctypes.c_int64), ctypes.c_size_t]
        lib.axon_start_nrt_profile.restype = ctypes.c_int64
        lib.axon_stop_nrt_profile.argtypes = [ctypes.c_char_p]
        lib.axon_stop_nrt_profile.restype = ctypes.c_int64

        @contextlib.contextmanager
        def hook(output_dir, device_ids):
            import jax
            jax.devices()
            if device_ids:
                ids = (ctypes.c_int64 * len(device_ids))(*device_ids)
                rc = lib.axon_start_nrt_profile(ids, len(device_ids))
            else:
                rc = lib.axon_start_nrt_profile(None, 0)
            if rc != 0:
                raise RuntimeError("axon_start_nrt_profile rc=%d" % rc)
            try:
                yield
            finally:
                n = lib.axon_stop_nrt_profile(str(output_dir).encode())
                print("profile: %d file(s) written to %s" % (n, output_dir),
                      file=sys.stderr)

        return hook

    hook = _make_hook()
    mod = types.ModuleType("antenv.axon_hooks")
    mod.get_axon_ntff_profile_hook = lambda: hook
    mod.set_axon_ntff_profile_hook = lambda h: None
    sys.modules["antenv.axon_hooks"] = mod


def build_graph(T):
    """Build the per-core Bass graph for T node tiles."""
    nc = bacc.Bacc()
    dp = nc.declare_dram_parameter
    hce_ext = dp("hce", [T, 128, 3 * TILE_E], F32, isOutput=False)
    sfdf_ext = dp("sfdf", [T, G, 2 * TILE_E], F32, isOutput=False)
    oh4_ext = dp("oh4", [4, T * TILE_E], BF16, isOutput=False)
    dstT_ext = dp("dstT", [128, T * CPT], F32, isOutput=False)
    welT_ext = dp("welT", [G, 128], F32, isOutput=False)
    wa_ext = dp("wa", [G, 4], F32, isOutput=False)
    bel_ext = dp("belB", [4, 128], F32, isOutput=False)
    wg_ext = {}
    for x in "fiuo":
        wg_ext[x] = dp("w%sT" % x, [2 * H, 128], F32, isOutput=False)
    bias_ext = {}
    for x in "fiuo":
        bias_ext[x] = (dp("bW%s" % x, [H], F32, isOutput=False),
                       dp("b%s" % x, [H], F32, isOutput=False))
    out_ext = dp("outT", [128, T * 2 * H], F32, isOutput=True)

    with tile.TileContext(nc) as tc:
        cst = tc.alloc_tile_pool(name="cst", bufs=1)
        pin = tc.alloc_tile_pool(name="pin", bufs=3)
        pcv = tc.alloc_tile_pool(name="pcv", bufs=2)
        pnd = tc.alloc_tile_pool(name="pnd", bufs=2)
        pacc = tc.alloc_tile_pool(name="pacc", bufs=2, space="PSUM")
        pmm = tc.alloc_tile_pool(name="pmm", bufs=2, space="PSUM")

        # -- setup: constants -----------------------------------------------
        welT_sb = cst.tile([G, 128], F32)
        nc.sync.dma_start(out=welT_sb[:], in_=welT_ext[:])
        wa_sb = cst.tile([G, 4], F32)
        nc.sync.dma_start(out=wa_sb[:], in_=wa_ext[:])
        belr = cst.tile([4, 128], F32)
        nc.sync.dma_start(out=belr[:], in_=bel_ext[:])
        t2p = pmm.tile([4, 128], F32, tag="mm")
        nc.tensor.matmul(out=t2p[:], lhsT=wa_sb[:], rhs=welT_sb[:],
                         start=True, stop=True)
        wel_b16 = cst.tile([G, 128], BF16)
        nc.vector.tensor_copy(out=wel_b16[:], in_=welT_sb[:])
        t4b = cst.tile([4, 128], BF16)
        nc.vector.tensor_tensor(out=t4b[:], in0=t2p[:],
                                in1=belr[:], op=mybir.AluOpType.add)
        wtcomb = cst.tile([G + 4, 128], BF16)
        nc.sync.dma_start(out=wtcomb[0:G, :], in_=wel_b16[:])
        nc.sync.dma_start(out=wtcomb[G:G + 4, :], in_=t4b[:])

        wg = {}
        for x in "fiuo":
            stg = cst.tile([128, 128], F32, tag="wstg_%s" % x)
            nc.sync.dma_start(out=stg[:], in_=wg_ext[x][0:128, :])
            wa_t = cst.tile([128, 128], BF16, tag="wg_%s_a" % x)
            nc.vector.tensor_copy(out=wa_t[:], in_=stg[:])
            stg2 = cst.tile([128, 128], F32, tag="wstg2_%s" % x)
            nc.sync.dma_start(out=stg2[:], in_=wg_ext[x][128:256, :])
            wb_t = cst.tile([128, 128], BF16, tag="wg_%s_b" % x)
            nc.vector.tensor_copy(out=wb_t[:], in_=stg2[:])
            wg[x] = (wa_t, wb_t)

        bias = {}
        for x in "fiuo":
            b1 = cst.tile([128, 1], F32, tag="b1_%s" % x)
            nc.sync.dma_start(out=b1[:], in_=bias_ext[x][0][:, None])
            b2 = cst.tile([128, 1], F32, tag="b2_%s" % x)
            nc.sync.dma_start(out=b2[:], in_=bias_ext[x][1][:, None])
            bs = cst.tile([128, 1], F32, tag="bs_%s" % x)
            nc.vector.tensor_tensor(out=bs[:], in0=b1[:], in1=b2[:],
                                    op=mybir.AluOpType.add)
            bias[x] = bs

        it32 = cst.tile([128, TILE_E], mybir.dt.int32)
        nc.gpsimd.iota(out=it32[:], pattern=[[0, CPT], [1, CHUNK]], base=0,
                       channel_multiplier=0)
        iotaF = cst.tile([128, TILE_E], F32)
        nc.vector.tensor_copy(out=iotaF[:], in_=it32[:])

        dstT = cst.tile([128, T * CPT], F32)
        nc.sync.dma_start(out=dstT[:], in_=dstT_ext[:])

        # -- main loop over node tiles --------------------------------------
        for t in range(T):
            hce = pin.tile([128, 3 * TILE_E], F32, tag="hce")
            nc.sync.dma_start(out=hce[:], in_=hce_ext[t])
            h4 = hce[:, 0:TILE_E]
            c4 = hce[:, TILE_E:2 * TILE_E]
            e4 = hce[:, 2 * TILE_E:3 * TILE_E]
            sfdf = pin.tile([G, 2 * TILE_E], F32, tag="sfdf")
            nc.sync.dma_start(out=sfdf[:], in_=sfdf_ext[t])
            B68 = pcv.tile([G + 4, TILE_E], BF16, tag="B68")
            nc.sync.dma_start(out=B68[G:G + 4, :],
                              in_=oh4_ext[:, t * TILE_E:(t + 1) * TILE_E])
            nc.vector.tensor_tensor(out=B68[0:G, :], in0=sfdf[:, 0:TILE_E],
                                    in1=sfdf[:, TILE_E:2 * TILE_E],
                                    op=mybir.AluOpType.mult)
            M4 = pcv.tile([128, TILE_E], BF16, tag="M4")
            nc.vector.tensor_tensor(
                out=M4[:].rearrange("p (c j) -> p c j", c=CPT),
                in0=iotaF[:].rearrange("p (c j) -> p c j", c=CPT),
                in1=dstT[:, t * CPT:(t + 1) * CPT, None]
                    .to_broadcast([128, CPT, CHUNK]),
                op=mybir.AluOpType.is_equal)
            e4b = pcv.tile([128, TILE_E], BF16, tag="e4b")
            nc.scalar.activation(out=e4b[:], in_=e4,
                                 func=mybir.ActivationFunctionType.Copy)
            c4b = pcv.tile([128, TILE_E], BF16, tag="c4b")
            nc.gpsimd.tensor_copy(out=c4b[:], in_=c4)

            ew4 = pmm.tile([128, TILE_E], F32, tag="ew4", space="PSUM")
            for c in range(CPT):
                sl = slice(c * CHUNK, (c + 1) * CHUNK)
                nc.tensor.matmul(out=ew4[:, sl], lhsT=B68[:, sl],
                                 rhs=wtcomb[:], start=True, stop=True)
            hw4 = pcv.tile([128, TILE_E], BF16, tag="hw4")
            nc.vector.tensor_tensor(out=hw4[:], in0=h4, in1=ew4[:],
                                    op=mybir.AluOpType.mult)

            hs12 = pacc.tile([128, 256], F32, tag="hs12", space="PSUM")
            cs = pacc.tile([128, 128], F32, tag="cs", space="PSUM")
            # keep each PSUM accumulation group contiguous -- interleaving
            # groups within one bank breaks has_written accumulate semantics
            for lhs4, dst_ap in ((hw4, hs12[:, 0:128]),
                                 (e4b, hs12[:, 128:256]),
                                 (c4b, cs[:])):
                for c in range(CPT):
                    sl = slice(c * CHUNK, (c + 1) * CHUNK)
                    nc.tensor.matmul(out=dst_ap, lhsT=lhs4[:, sl],
                                     rhs=M4[:, sl], start=(c == 0),
                                     stop=(c == CPT - 1))

            hsab = pnd.tile([128, 256], BF16, tag="hsab")
            nc.vector.tensor_copy(out=hsab[:], in_=hs12[:])

            gate = {}
            for x, fn in (("f", "Sigmoid"), ("i", "Sigmoid"),
                          ("u", "Tanh"), ("o", "Sigmoid")):
                gp = pmm.tile([128, 128], F32, tag="mm", space="PSUM")
                nc.tensor.matmul(out=gp[:], lhsT=wg[x][0][:],
                                 rhs=hsab[:, 0:128], start=True, stop=False)
                nc.tensor.matmul(out=gp[:], lhsT=wg[x][1][:],
                                 rhs=hsab[:, 128:256], start=False, stop=True)
                gs = pnd.tile([128, 128], F32, tag="g_%s" % x)
                nc.scalar.activation(
                    out=gs[:], in_=gp[:],
                    func=getattr(mybir.ActivationFunctionType, fn),
                    bias=bias[x][:])
                gate[x] = gs

            hc = pnd.tile([128, 2 * H], F32, tag="hc")
            ct = pnd.tile([128, 128], F32, tag="ct")
            nc.vector.tensor_tensor(out=ct[:], in0=gate["f"][:], in1=cs[:],
                                    op=mybir.AluOpType.mult)
            iu = pnd.tile([128, 128], F32, tag="iu")
            nc.vector.tensor_tensor(out=iu[:], in0=gate["i"][:],
                                    in1=gate["u"][:], op=mybir.AluOpType.mult)
            nc.vector.tensor_tensor(out=hc[:, 128:256], in0=iu[:], in1=ct[:],
                                    op=mybir.AluOpType.add)
            th = pnd.tile([128, 128], F32, tag="th")
            nc.scalar.activation(out=th[:], in_=hc[:, 128:256],
                                 func=mybir.ActivationFunctionType.Tanh)
            nc.vector.tensor_tensor(out=hc[:, 0:128], in0=gate["o"][:],
                                    in1=th[:], op=mybir.AluOpType.mult)
            nc.gpsimd.dma_start(
                out=out_ext[:, t * 2 * H:(t + 1) * 2 * H], in_=hc[:])

        for p in (pmm, pacc, pnd, pcv, pin, cst):
            p.release()
    nc.finalize()
    return nc


def plan_tiles(dst_local, npc):
    """Greedy node tiling: <=128 nodes and <=TILE_E edges per tile.
    Returns list of (n0, n1, e0, e1) using sorted-edge offsets."""
    cnt = np.bincount(dst_local, minlength=npc)
    cum = np.concatenate([[0], np.cumsum(cnt)])
    tiles = []
    s = 0
    while s < npc:
        hi = min(s + 128, npc)
        m = int(np.searchsorted(cum, cum[s] + TILE_E, side="right")) - 1
        m = max(s + 1, min(hi, m))
        tiles.append((s, m, int(cum[s]), int(cum[m])))
        s = m
    return tiles


def prep_core(k, h_src, c_src, embed_dst, src_f, dst_f, etype, dst, T=None):
    """Build one core's padded, tiled input arrays."""
    lo = k * NPC
    sel = np.nonzero((dst >= lo) & (dst < lo + NPC))[0]
    dl = (dst[sel] - lo).astype(np.int64)
    order = np.argsort(dl, kind="stable")
    eidx = sel[order]
    dls = dl[order]
    tiles = plan_tiles(dls, NPC)
    Tk = len(tiles)
    if T is None:
        T = Tk
    assert Tk <= T
    ES = T * TILE_E
    src_slot = np.full(ES, -1, dtype=np.int64)
    dst_slot = np.full(ES, -1.0, dtype=np.float32)
    for t, (n0, n1, e0, e1) in enumerate(tiles):
        ne = e1 - e0
        assert ne <= TILE_E and n1 - n0 <= 128
        src_slot[t * TILE_E:t * TILE_E + ne] = eidx[e0:e1]
        dst_slot[t * TILE_E:t * TILE_E + ne] = (dls[e0:e1] - n0).astype(np.float32)
    val = src_slot >= 0
    gi = src_slot[val]

    def pad_rows(a, w):
        out = np.zeros((ES, w), dtype=np.float32)
        out[val] = a[gi]
        return out

    def chunk_layout(a):
        # [ES, H] -> [T, 128, TILE_E] with slot (c*128+p) at [t, p, c*128:...]
        return a.reshape(T, CPT, CHUNK, H).transpose(0, 2, 1, 3) \
                .reshape(T, 128, TILE_E)

    hp = chunk_layout(pad_rows(h_src, H))
    cp = chunk_layout(pad_rows(c_src, H))
    ep = chunk_layout(pad_rows(embed_dst, H))
    hce = np.ascontiguousarray(np.concatenate([hp, cp, ep], axis=2))
    sf = pad_rows(src_f, G).reshape(T, TILE_E, G).transpose(0, 2, 1)
    df = pad_rows(dst_f, G).reshape(T, TILE_E, G).transpose(0, 2, 1)
    sfdf = np.ascontiguousarray(np.concatenate([sf, df], axis=2))
    oh = np.zeros((ES, 4), dtype=np.float32)
    oh[val, etype[gi]] = 1.0
    oh[val, 3] = 1.0
    ohT = np.ascontiguousarray(oh.T).astype(bf16_np)
    dstT = np.ascontiguousarray(dst_slot.reshape(T * CPT, CHUNK).T)
    return {"hce": hce, "sfdf": sfdf, "oh4": ohT, "dstT": dstT}, tiles, Tk


def _belB(b_el):
    out = np.zeros((4, 128), dtype=np.float32)
    out[3] = b_el
    return out


_graph_cache = {}


def kernel(**inputs):
    h_src = np.asarray(inputs["h_src"], dtype=np.float32)
    c_src = np.asarray(inputs["c_src"], dtype=np.float32)
    embed_dst = np.asarray(inputs["embed_dst"], dtype=np.float32)
    src_f = np.asarray(inputs["src_node_feat"], dtype=np.float32)
    dst_f = np.asarray(inputs["dst_node_feat"], dtype=np.float32)
    etype = np.asarray(inputs["edge_type_idx"]).astype(np.int64)
    dst = np.asarray(inputs["dst_idx"]).astype(np.int64)

    weights = {
        "welT": np.ascontiguousarray(np.asarray(inputs["W_el"], np.float32).T),
        "wa": np.ascontiguousarray(np.concatenate(
            [np.asarray(inputs["W_eoh"], np.float32),
             np.asarray(inputs["b_eoh"], np.float32)[:, None]], axis=1)),
        "belB": _belB(np.asarray(inputs["b_el"], np.float32)),
    }
    for x, wn, bwn, bn in (("f", "Wf", "bWf", "bf"), ("i", "Wi", "bWi", "bi"),
                           ("u", "Wu", "bWu", "bu"), ("o", "Wo", "bWo", "bo")):
        weights["w%sT" % x] = np.ascontiguousarray(
            np.asarray(inputs[wn], np.float32).T)
        weights["bW%s" % x] = np.asarray(inputs[bwn], np.float32)
        weights["b%s" % x] = np.asarray(inputs[bn], np.float32)

    planned = []
    for k in range(NCORES):
        lo = k * NPC
        sel = np.nonzero((dst >= lo) & (dst < lo + NPC))[0]
        dl = np.sort((dst[sel] - lo).astype(np.int64))
        planned.append(plan_tiles(dl, NPC))
    T = max(len(p) for p in planned)

    in_maps = []
    tiles_all = []
    for k in range(NCORES):
        m, tiles, _ = prep_core(k, h_src, c_src, embed_dst, src_f, dst_f,
                                etype, dst, T=T)
        m.update(weights)
        in_maps.append(m)
        tiles_all.append(tiles)

    if T not in _graph_cache:
        _graph_cache[T] = build_graph(T)
    nc = _graph_cache[T]

    if TRACE:
        _install_axon_hook()
    res = run_bass_kernel_spmd(nc, in_maps, list(range(NCORES)), trace=TRACE)
    LAST["res"] = res

    out = np.empty((N, 2 * H), dtype=np.float32)
    for k in range(NCORES):
        outT = np.asarray(res.results[k]["outT"])
        for t, (n0, n1, _, _) in enumerate(tiles_all[k]):
            nn = n1 - n0
            base = k * NPC
            out[base + n0:base + n1, 0:H] = outT[:, t * 2 * H:t * 2 * H + nn].T
            out[base + n0:base + n1, H:2 * H] = \
                outT[:, t * 2 * H + H:t * 2 * H + H + nn].T
    return out


# revision 12
# speedup vs baseline: 1.2847x; 1.0376x over previous
"""ChildSumTreeLSTMCell on 8 Trainium2 NeuronCores.

Strategy: sort edges by destination node on the host (index preprocessing as
part of sharding), partition nodes contiguously across the 8 cores so every
core's segment sums are fully local -- zero collectives.  On each core, edges
are packed into node tiles (<=128 nodes, <=512 edges each, 4 chunks of 128
edge slots).  Segment sums become matmuls against a 0/1 membership matrix M
built on-device from the destination indices; the forget-gate gather
f[dst] * c_src factorizes to f * segment_sum(c_src), removing the second
scatter entirely.  All matmuls run in bf16 (inputs stay f32 in HBM).

v2: packed DMA streams (hce = h|c|e in one 768KB transfer, sfdf merged,
single merged [h|c] output), DMA issue spread across sync/gpsimd queues,
one-shot M build via free-dim broadcast, merged PSUM evacuation.
"""

import sys

for _p in ("/opt/trn_rl_repo", "/root/.axon_site/_ro/trn_rl_repo"):
    if _p not in sys.path:
        sys.path.append(_p)

import numpy as np
import ml_dtypes

import concourse.bacc as bacc
import concourse.mybir as mybir
import concourse.tile as tile
from concourse.bass_utils import run_bass_kernel_spmd

F32 = mybir.dt.float32
BF16 = mybir.dt.bfloat16

E = 500_000
N = 125_000
H = 128
G = 64
NCORES = 8
NPC = N // NCORES          # nodes per core
CHUNK = 128                # edges per chunk
CPT = 4                    # chunks per tile
TILE_E = CHUNK * CPT       # edge slots per tile
bf16_np = ml_dtypes.bfloat16

TRACE = False              # set by test.py to capture an NTFF profile
LAST = {}                  # last run's BassKernelResults


def _install_axon_hook():
    import types, contextlib, ctypes

    def _make_hook(so_path="/opt/axon/libaxon_pjrt.so"):
        lib = ctypes.CDLL(so_path)
        if not hasattr(lib, "axon_start_nrt_profile"):
            return None
        lib.axon_start_nrt_profile.argtypes = [
            ctypes.POINTER(ctypes.c_int64), ctypes.c_size_t]
        lib.axon_start_nrt_profile.restype = ctypes.c_int64
        lib.axon_stop_nrt_profile.argtypes = [ctypes.c_char_p]
        lib.axon_stop_nrt_profile.restype = ctypes.c_int64

        @contextlib.contextmanager
        def hook(output_dir, device_ids):
            import jax
            jax.devices()
            if device_ids:
                ids = (ctypes.c_int64 * len(device_ids))(*device_ids)
                rc = lib.axon_start_nrt_profile(ids, len(device_ids))
            else:
                rc = lib.axon_start_nrt_profile(None, 0)
            if rc != 0:
                raise RuntimeError("axon_start_nrt_profile rc=%d" % rc)
            try:
                yield
            finally:
                n = lib.axon_stop_nrt_profile(str(output_dir).encode())
                print("profile: %d file(s) written to %s" % (n, output_dir),
                      file=sys.stderr)

        return hook

    hook = _make_hook()
    mod = types.ModuleType("antenv.axon_hooks")
    mod.get_axon_ntff_profile_hook = lambda: hook
    mod.set_axon_ntff_profile_hook = lambda h: None
    sys.modules["antenv.axon_hooks"] = mod


def build_graph(T):
    """Build the per-core Bass graph for T node tiles."""
    nc = bacc.Bacc()
    dp = nc.declare_dram_parameter
    hce_ext = dp("hce", [T, 128, 3 * TILE_E], F32, isOutput=False)
    sfdf_ext = dp("sfdf", [T, G, 2 * TILE_E], F32, isOutput=False)
    oh4_ext = dp("oh4", [4, T * TILE_E], BF16, isOutput=False)
    dstT_ext = dp("dstT", [128, T * CPT], F32, isOutput=False)
    welT_ext = dp("welT", [G, 128], F32, isOutput=False)
    wa_ext = dp("wa", [G, 4], F32, isOutput=False)
    bel_ext = dp("belB", [4, 128], F32, isOutput=False)
    wg_ext = {}
    for x in "fiuo":
        wg_ext[x] = dp("w%sT" % x, [2 * H, 128], F32, isOutput=False)
    bias_ext = {}
    for x in "fiuo":
        bias_ext[x] = (dp("bW%s" % x, [H], F32, isOutput=False),
                       dp("b%s" % x, [H], F32, isOutput=False))
    out_ext = dp("outT", [128, T * 2 * H], F32, isOutput=True)

    with tile.TileContext(nc) as tc:
        cst = tc.alloc_tile_pool(name="cst", bufs=1)
        pin = tc.alloc_tile_pool(name="pin", bufs=3)
        pcv = tc.alloc_tile_pool(name="pcv", bufs=2)
        pnd = tc.alloc_tile_pool(name="pnd", bufs=2)
        pacc = tc.alloc_tile_pool(name="pacc", bufs=2, space="PSUM")
        pmm = tc.alloc_tile_pool(name="pmm", bufs=2, space="PSUM")

        # -- setup: constants -----------------------------------------------
        welT_sb = cst.tile([G, 128], F32)
        nc.sync.dma_start(out=welT_sb[:], in_=welT_ext[:])
        wa_sb = cst.tile([G, 4], F32)
        nc.sync.dma_start(out=wa_sb[:], in_=wa_ext[:])
        belr = cst.tile([4, 128], F32)
        nc.sync.dma_start(out=belr[:], in_=bel_ext[:])
        t2p = pmm.tile([4, 128], F32, tag="mm")
        nc.tensor.matmul(out=t2p[:], lhsT=wa_sb[:], rhs=welT_sb[:],
                         start=True, stop=True)
        wel_b16 = cst.tile([G, 128], BF16)
        nc.vector.tensor_copy(out=wel_b16[:], in_=welT_sb[:])
        t4b = cst.tile([4, 128], BF16)
        nc.vector.tensor_tensor(out=t4b[:], in0=t2p[:],
                                in1=belr[:], op=mybir.AluOpType.add)
        wtcomb = cst.tile([G + 4, 128], BF16)
        nc.sync.dma_start(out=wtcomb[0:G, :], in_=wel_b16[:])
        nc.sync.dma_start(out=wtcomb[G:G + 4, :], in_=t4b[:])

        wg = {}
        for x in "fiuo":
            stg = cst.tile([128, 128], F32, tag="wstg_%s" % x)
            nc.sync.dma_start(out=stg[:], in_=wg_ext[x][0:128, :])
            wa_t = cst.tile([128, 128], BF16, tag="wg_%s_a" % x)
            nc.vector.tensor_copy(out=wa_t[:], in_=stg[:])
            stg2 = cst.tile([128, 128], F32, tag="wstg2_%s" % x)
            nc.sync.dma_start(out=stg2[:], in_=wg_ext[x][128:256, :])
            wb_t = cst.tile([128, 128], BF16, tag="wg_%s_b" % x)
            nc.vector.tensor_copy(out=wb_t[:], in_=stg2[:])
            wg[x] = (wa_t, wb_t)

        bias = {}
        for x in "fiuo":
            b1 = cst.tile([128, 1], F32, tag="b1_%s" % x)
            nc.sync.dma_start(out=b1[:], in_=bias_ext[x][0][:, None])
            b2 = cst.tile([128, 1], F32, tag="b2_%s" % x)
            nc.sync.dma_start(out=b2[:], in_=bias_ext[x][1][:, None])
            bs = cst.tile([128, 1], F32, tag="bs_%s" % x)
            nc.vector.tensor_tensor(out=bs[:], in0=b1[:], in1=b2[:],
                                    op=mybir.AluOpType.add)
            bias[x] = bs

        it32 = cst.tile([128, 2 * TILE_E], mybir.dt.int32)
        nc.gpsimd.iota(out=it32[:], pattern=[[0, 2 * CPT], [1, CHUNK]], base=0,
                       channel_multiplier=0)
        iotaF = cst.tile([128, 2 * TILE_E], F32)
        nc.vector.tensor_copy(out=iotaF[:], in_=it32[:])

        dstT = cst.tile([128, T * CPT], F32)
        nc.sync.dma_start(out=dstT[:], in_=dstT_ext[:])

        # -- main loop: groups of 2 node tiles ------------------------------
        assert T % 2 == 0
        AF = mybir.ActivationFunctionType
        for g in range(T // 2):
            # [128, 2, 1536] = both tiles' h|c|e streams, one DMA
            hce = pin.tile([128, 2, 3 * TILE_E], F32, tag="hce")
            nc.sync.dma_start(
                out=hce[:], in_=hce_ext[2 * g:2 * g + 2]
                .rearrange("t p f -> p t f"))
            sfdf = pin.tile([G, 2, 2 * TILE_E], F32, tag="sfdf")
            nc.scalar.dma_start(
                out=sfdf[:], in_=sfdf_ext[2 * g:2 * g + 2]
                .rearrange("t p f -> p t f"))
            B68 = pcv.tile([G + 4, 2 * TILE_E], BF16, tag="B68")
            nc.sync.dma_start(
                out=B68[G:G + 4, :],
                in_=oh4_ext[:, g * 2 * TILE_E:(g + 1) * 2 * TILE_E])
            nc.vector.tensor_tensor(
                out=B68[0:G, :].rearrange("p (t f) -> p t f", t=2),
                in0=sfdf[:, :, 0:TILE_E], in1=sfdf[:, :, TILE_E:2 * TILE_E],
                op=mybir.AluOpType.mult)
            M4 = pcv.tile([128, 2 * TILE_E], BF16, tag="M4")
            nc.vector.tensor_tensor(
                out=M4[:].rearrange("p (c j) -> p c j", c=2 * CPT),
                in0=iotaF[:].rearrange("p (c j) -> p c j", c=2 * CPT),
                in1=dstT[:, g * 2 * CPT:(g + 1) * 2 * CPT, None]
                    .to_broadcast([128, 2 * CPT, CHUNK]),
                op=mybir.AluOpType.is_equal)
            e4b = pcv.tile([128, 2 * TILE_E], BF16, tag="e4b")
            nc.scalar.activation(
                out=e4b[:].rearrange("p (t f) -> p t f", t=2),
                in_=hce[:, :, 2 * TILE_E:3 * TILE_E], func=AF.Copy)
            c4b = pcv.tile([128, 2 * TILE_E], BF16, tag="c4b")
            nc.gpsimd.tensor_copy(
                out=c4b[:].rearrange("p (t f) -> p t f", t=2),
                in_=hce[:, :, TILE_E:2 * TILE_E])

            ewb = pcv.tile([128, 2 * TILE_E], F32, tag="ewb")
            for tl in range(2):
                ew4 = pmm.tile([128, TILE_E], F32, tag="ew4", space="PSUM")
                for c in range(CPT):
                    lsl = slice(tl * TILE_E + c * CHUNK,
                                tl * TILE_E + (c + 1) * CHUNK)
                    nc.tensor.matmul(out=ew4[:, c * CHUNK:(c + 1) * CHUNK],
                                     lhsT=B68[:, lsl], rhs=wtcomb[:],
                                     start=True, stop=True)
                nc.scalar.activation(
                    out=ewb[:, tl * TILE_E:(tl + 1) * TILE_E], in_=ew4[:],
                    func=AF.Copy)
            hw4 = pcv.tile([128, 2 * TILE_E], BF16, tag="hw4")
            nc.vector.tensor_tensor(
                out=hw4[:].rearrange("p (t f) -> p t f", t=2),
                in0=hce[:, :, 0:TILE_E],
                in1=ewb[:].rearrange("p (t f) -> p t f", t=2),
                op=mybir.AluOpType.mult)

            # hs12 regions: [0:128]=t0_a  [128:256]=t1_a
            #               [256:384]=t0_b [384:512]=t1_b
            hs12 = pacc.tile([128, 4 * 128], F32, tag="hs12", space="PSUM")
            cs = pacc.tile([128, 256], F32, tag="cs", space="PSUM")
            for tl in range(2):
                for lhs4, dst_ap in (
                        (hw4, hs12[:, tl * 128:(tl + 1) * 128]),
                        (e4b, hs12[:, 256 + tl * 128:256 + (tl + 1) * 128]),
                        (c4b, cs[:, tl * 128:(tl + 1) * 128])):
                    for c in range(CPT):
                        sl = slice(tl * TILE_E + c * CHUNK,
                                   tl * TILE_E + (c + 1) * CHUNK)
                        nc.tensor.matmul(out=dst_ap, lhsT=lhs4[:, sl],
                                         rhs=M4[:, sl], start=(c == 0),
                                         stop=(c == CPT - 1))

            hsab = pnd.tile([128, 512], BF16, tag="hsab")
            nc.vector.tensor_copy(out=hsab[:], in_=hs12[:])
            css = pnd.tile([128, 256], F32, tag="css")
            nc.scalar.activation(out=css[:], in_=cs[:], func=AF.Copy)

            gate = {}
            for x, fn in (("f", "Sigmoid"), ("i", "Sigmoid"),
                          ("u", "Tanh"), ("o", "Sigmoid")):
                gp = pmm.tile([128, 256], F32, tag="mm", space="PSUM")
                nc.tensor.matmul(out=gp[:], lhsT=wg[x][0][:],
                                 rhs=hsab[:, 0:256], start=True, stop=False)
                nc.tensor.matmul(out=gp[:], lhsT=wg[x][1][:],
                                 rhs=hsab[:, 256:512], start=False, stop=True)
                gs = pnd.tile([128, 256], F32, tag="g_%s" % x)
                nc.scalar.activation(out=gs[:], in_=gp[:],
                                     func=getattr(AF, fn), bias=bias[x][:])
                gate[x] = gs

            # hc layout: [t0_h t0_c t1_h t1_c]
            hc = pnd.tile([128, 512], F32, tag="hc")
            hc_t = hc[:].rearrange("p (t x) -> p t x", t=2)
            ct = pnd.tile([128, 256], F32, tag="ct")
            nc.vector.tensor_tensor(out=ct[:], in0=gate["f"][:], in1=css[:],
                                    op=mybir.AluOpType.mult)
            iu = pnd.tile([128, 256], F32, tag="iu")
            nc.vector.tensor_tensor(out=iu[:], in0=gate["i"][:],
                                    in1=gate["u"][:], op=mybir.AluOpType.mult)
            c2 = iu[:].rearrange("p (t x) -> p t x", t=2)
            nc.vector.tensor_tensor(out=hc_t[:, :, 128:256], in0=c2,
                                    in1=ct[:].rearrange("p (t x) -> p t x", t=2),
                                    op=mybir.AluOpType.add)
            th = pnd.tile([128, 256], F32, tag="th")
            nc.scalar.activation(out=th[:].rearrange("p (t x) -> p t x", t=2),
                                 in_=hc_t[:, :, 128:256], func=AF.Tanh)
            nc.vector.tensor_tensor(out=hc_t[:, :, 0:128],
                                    in0=gate["o"][:].rearrange(
                                        "p (t x) -> p t x", t=2),
                                    in1=th[:].rearrange("p (t x) -> p t x", t=2),
                                    op=mybir.AluOpType.mult)
            nc.gpsimd.dma_start(
                out=out_ext[:, g * 512:(g + 1) * 512], in_=hc[:])

        for p in (pmm, pacc, pnd, pcv, pin, cst):
            p.release()
    nc.finalize()
    return nc


def plan_tiles(dst_local, npc):
    """Greedy node tiling: <=128 nodes and <=TILE_E edges per tile.
    Returns list of (n0, n1, e0, e1) using sorted-edge offsets."""
    cnt = np.bincount(dst_local, minlength=npc)
    cum = np.concatenate([[0], np.cumsum(cnt)])
    tiles = []
    s = 0
    while s < npc:
        hi = min(s + 128, npc)
        m = int(np.searchsorted(cum, cum[s] + TILE_E, side="right")) - 1
        m = max(s + 1, min(hi, m))
        tiles.append((s, m, int(cum[s]), int(cum[m])))
        s = m
    return tiles


def prep_core(k, h_src, c_src, embed_dst, src_f, dst_f, etype, dst, T=None):
    """Build one core's padded, tiled input arrays."""
    lo = k * NPC
    sel = np.nonzero((dst >= lo) & (dst < lo + NPC))[0]
    dl = (dst[sel] - lo).astype(np.int64)
    order = np.argsort(dl, kind="stable")
    eidx = sel[order]
    dls = dl[order]
    tiles = plan_tiles(dls, NPC)
    Tk = len(tiles)
    if T is None:
        T = Tk
    assert Tk <= T
    ES = T * TILE_E
    src_slot = np.full(ES, -1, dtype=np.int64)
    dst_slot = np.full(ES, -1.0, dtype=np.float32)
    for t, (n0, n1, e0, e1) in enumerate(tiles):
        ne = e1 - e0
        assert ne <= TILE_E and n1 - n0 <= 128
        src_slot[t * TILE_E:t * TILE_E + ne] = eidx[e0:e1]
        dst_slot[t * TILE_E:t * TILE_E + ne] = (dls[e0:e1] - n0).astype(np.float32)
    val = src_slot >= 0
    gi = src_slot[val]

    def pad_rows(a, w):
        out = np.zeros((ES, w), dtype=np.float32)
        out[val] = a[gi]
        return out

    def chunk_layout(a):
        # [ES, H] -> [T, 128, TILE_E] with slot (c*128+p) at [t, p, c*128:...]
        return a.reshape(T, CPT, CHUNK, H).transpose(0, 2, 1, 3) \
                .reshape(T, 128, TILE_E)

    hp = chunk_layout(pad_rows(h_src, H))
    cp = chunk_layout(pad_rows(c_src, H))
    ep = chunk_layout(pad_rows(embed_dst, H))
    hce = np.ascontiguousarray(np.concatenate([hp, cp, ep], axis=2))
    sf = pad_rows(src_f, G).reshape(T, TILE_E, G).transpose(0, 2, 1)
    df = pad_rows(dst_f, G).reshape(T, TILE_E, G).transpose(0, 2, 1)
    sfdf = np.ascontiguousarray(np.concatenate([sf, df], axis=2))
    oh = np.zeros((ES, 4), dtype=np.float32)
    oh[val, etype[gi]] = 1.0
    oh[val, 3] = 1.0
    ohT = np.ascontiguousarray(oh.T).astype(bf16_np)
    dstT = np.ascontiguousarray(dst_slot.reshape(T * CPT, CHUNK).T)
    return {"hce": hce, "sfdf": sfdf, "oh4": ohT, "dstT": dstT}, tiles, Tk


def _belB(b_el):
    out = np.zeros((4, 128), dtype=np.float32)
    out[3] = b_el
    return out


_graph_cache = {}


def kernel(**inputs):
    h_src = np.asarray(inputs["h_src"], dtype=np.float32)
    c_src = np.asarray(inputs["c_src"], dtype=np.float32)
    embed_dst = np.asarray(inputs["embed_dst"], dtype=np.float32)
    src_f = np.asarray(inputs["src_node_feat"], dtype=np.float32)
    dst_f = np.asarray(inputs["dst_node_feat"], dtype=np.float32)
    etype = np.asarray(inputs["edge_type_idx"]).astype(np.int64)
    dst = np.asarray(inputs["dst_idx"]).astype(np.int64)

    weights = {
        "welT": np.ascontiguousarray(np.asarray(inputs["W_el"], np.float32).T),
        "wa": np.ascontiguousarray(np.concatenate(
            [np.asarray(inputs["W_eoh"], np.float32),
             np.asarray(inputs["b_eoh"], np.float32)[:, None]], axis=1)),
        "belB": _belB(np.asarray(inputs["b_el"], np.float32)),
    }
    for x, wn, bwn, bn in (("f", "Wf", "bWf", "bf"), ("i", "Wi", "bWi", "bi"),
                           ("u", "Wu", "bWu", "bu"), ("o", "Wo", "bWo", "bo")):
        weights["w%sT" % x] = np.ascontiguousarray(
            np.asarray(inputs[wn], np.float32).T)
        weights["bW%s" % x] = np.asarray(inputs[bwn], np.float32)
        weights["b%s" % x] = np.asarray(inputs[bn], np.float32)

    planned = []
    for k in range(NCORES):
        lo = k * NPC
        sel = np.nonzero((dst >= lo) & (dst < lo + NPC))[0]
        dl = np.sort((dst[sel] - lo).astype(np.int64))
        planned.append(plan_tiles(dl, NPC))
    T = max(len(p) for p in planned)
    T += T % 2  # group-of-2 tiling needs even T

    in_maps = []
    tiles_all = []
    for k in range(NCORES):
        m, tiles, _ = prep_core(k, h_src, c_src, embed_dst, src_f, dst_f,
                                etype, dst, T=T)
        m.update(weights)
        in_maps.append(m)
        tiles_all.append(tiles)

    if T not in _graph_cache:
        _graph_cache[T] = build_graph(T)
    nc = _graph_cache[T]

    if TRACE:
        _install_axon_hook()
    res = run_bass_kernel_spmd(nc, in_maps, list(range(NCORES)), trace=TRACE)
    LAST["res"] = res

    out = np.empty((N, 2 * H), dtype=np.float32)
    for k in range(NCORES):
        outT = np.asarray(res.results[k]["outT"])
        for t, (n0, n1, _, _) in enumerate(tiles_all[k]):
            nn = n1 - n0
            base = k * NPC
            out[base + n0:base + n1, 0:H] = outT[:, t * 2 * H:t * 2 * H + nn].T
            out[base + n0:base + n1, H:2 * H] = \
                outT[:, t * 2 * H + H:t * 2 * H + H + nn].T
    return out


# revision 13
# speedup vs baseline: 1.3026x; 1.0139x over previous
"""ChildSumTreeLSTMCell on 8 Trainium2 NeuronCores.

Strategy: sort edges by destination node on the host (index preprocessing as
part of sharding), partition nodes contiguously across the 8 cores so every
core's segment sums are fully local -- zero collectives.  On each core, edges
are packed into node tiles (<=128 nodes, <=512 edges each, 4 chunks of 128
edge slots).  Segment sums become matmuls against a 0/1 membership matrix M
built on-device from the destination indices; the forget-gate gather
f[dst] * c_src factorizes to f * segment_sum(c_src), removing the second
scatter entirely.  All matmuls run in bf16 (inputs stay f32 in HBM).

v2: packed DMA streams (hce = h|c|e in one 768KB transfer, sfdf merged,
single merged [h|c] output), DMA issue spread across sync/gpsimd queues,
one-shot M build via free-dim broadcast, merged PSUM evacuation.
"""

import sys

for _p in ("/opt/trn_rl_repo", "/root/.axon_site/_ro/trn_rl_repo"):
    if _p not in sys.path:
        sys.path.append(_p)

import numpy as np
import ml_dtypes

import concourse.bacc as bacc
import concourse.mybir as mybir
import concourse.tile as tile
from concourse.bass_utils import run_bass_kernel_spmd

F32 = mybir.dt.float32
BF16 = mybir.dt.bfloat16

E = 500_000
N = 125_000
H = 128
G = 64
NCORES = 8
NPC = N // NCORES          # nodes per core
CHUNK = 128                # edges per chunk
CPT = 4                    # chunks per tile
TILE_E = CHUNK * CPT       # edge slots per tile
bf16_np = ml_dtypes.bfloat16

TRACE = False              # set by test.py to capture an NTFF profile
LAST = {}                  # last run's BassKernelResults


def _install_axon_hook():
    import types, contextlib, ctypes

    def _make_hook(so_path="/opt/axon/libaxon_pjrt.so"):
        lib = ctypes.CDLL(so_path)
        if not hasattr(lib, "axon_start_nrt_profile"):
            return None
        lib.axon_start_nrt_profile.argtypes = [
            ctypes.POINTER(ctypes.c_int64), ctypes.c_size_t]
        lib.axon_start_nrt_profile.restype = ctypes.c_int64
        lib.axon_stop_nrt_profile.argtypes = [ctypes.c_char_p]
        lib.axon_stop_nrt_profile.restype = ctypes.c_int64

        @contextlib.contextmanager
        def hook(output_dir, device_ids):
            import jax
            jax.devices()
            if device_ids:
                ids = (ctypes.c_int64 * len(device_ids))(*device_ids)
                rc = lib.axon_start_nrt_profile(ids, len(device_ids))
            else:
                rc = lib.axon_start_nrt_profile(None, 0)
            if rc != 0:
                raise RuntimeError("axon_start_nrt_profile rc=%d" % rc)
            try:
                yield
            finally:
                n = lib.axon_stop_nrt_profile(str(output_dir).encode())
                print("profile: %d file(s) written to %s" % (n, output_dir),
                      file=sys.stderr)

        return hook

    hook = _make_hook()
    mod = types.ModuleType("antenv.axon_hooks")
    mod.get_axon_ntff_profile_hook = lambda: hook
    mod.set_axon_ntff_profile_hook = lambda h: None
    sys.modules["antenv.axon_hooks"] = mod


def build_graph(T):
    """Build the per-core Bass graph for T node tiles."""
    nc = bacc.Bacc()
    dp = nc.declare_dram_parameter
    hce_ext = dp("hce", [T // 2, 128, 6 * TILE_E], F32, isOutput=False)
    sfdf_ext = dp("sfdf", [T // 2, G, 4 * TILE_E], F32, isOutput=False)
    oh4_ext = dp("oh4", [4, T * TILE_E], BF16, isOutput=False)
    dstT_ext = dp("dstT", [128, T * CPT], F32, isOutput=False)
    welT_ext = dp("welT", [G, 128], F32, isOutput=False)
    wa_ext = dp("wa", [G, 4], F32, isOutput=False)
    bel_ext = dp("belB", [4, 128], F32, isOutput=False)
    wg_ext = {}
    for x in "fiuo":
        wg_ext[x] = dp("w%sT" % x, [2 * H, 128], F32, isOutput=False)
    bias_ext = {}
    for x in "fiuo":
        bias_ext[x] = (dp("bW%s" % x, [H], F32, isOutput=False),
                       dp("b%s" % x, [H], F32, isOutput=False))
    out_ext = dp("outT", [128, T * 2 * H], F32, isOutput=True)

    with tile.TileContext(nc) as tc:
        cst = tc.alloc_tile_pool(name="cst", bufs=1)
        pin = tc.alloc_tile_pool(name="pin", bufs=4)
        pcv = tc.alloc_tile_pool(name="pcv", bufs=2)
        pnd = tc.alloc_tile_pool(name="pnd", bufs=2)
        pacc = tc.alloc_tile_pool(name="pacc", bufs=2, space="PSUM")
        pmm = tc.alloc_tile_pool(name="pmm", bufs=2, space="PSUM")

        # -- setup: constants -----------------------------------------------
        welT_sb = cst.tile([G, 128], F32)
        nc.sync.dma_start(out=welT_sb[:], in_=welT_ext[:])
        wa_sb = cst.tile([G, 4], F32)
        nc.sync.dma_start(out=wa_sb[:], in_=wa_ext[:])
        belr = cst.tile([4, 128], F32)
        nc.sync.dma_start(out=belr[:], in_=bel_ext[:])
        t2p = pmm.tile([4, 128], F32, tag="mm")
        nc.tensor.matmul(out=t2p[:], lhsT=wa_sb[:], rhs=welT_sb[:],
                         start=True, stop=True)
        wel_b16 = cst.tile([G, 128], BF16)
        nc.vector.tensor_copy(out=wel_b16[:], in_=welT_sb[:])
        t4b = cst.tile([4, 128], BF16)
        nc.vector.tensor_tensor(out=t4b[:], in0=t2p[:],
                                in1=belr[:], op=mybir.AluOpType.add)
        wtcomb = cst.tile([G + 4, 128], BF16)
        nc.sync.dma_start(out=wtcomb[0:G, :], in_=wel_b16[:])
        nc.sync.dma_start(out=wtcomb[G:G + 4, :], in_=t4b[:])

        wg = {}
        for x in "fiuo":
            stg = cst.tile([128, 128], F32, tag="wstg_%s" % x)
            nc.sync.dma_start(out=stg[:], in_=wg_ext[x][0:128, :])
            wa_t = cst.tile([128, 128], BF16, tag="wg_%s_a" % x)
            nc.vector.tensor_copy(out=wa_t[:], in_=stg[:])
            stg2 = cst.tile([128, 128], F32, tag="wstg2_%s" % x)
            nc.sync.dma_start(out=stg2[:], in_=wg_ext[x][128:256, :])
            wb_t = cst.tile([128, 128], BF16, tag="wg_%s_b" % x)
            nc.vector.tensor_copy(out=wb_t[:], in_=stg2[:])
            wg[x] = (wa_t, wb_t)

        bias = {}
        for x in "fiuo":
            b1 = cst.tile([128, 1], F32, tag="b1_%s" % x)
            nc.sync.dma_start(out=b1[:], in_=bias_ext[x][0][:, None])
            b2 = cst.tile([128, 1], F32, tag="b2_%s" % x)
            nc.sync.dma_start(out=b2[:], in_=bias_ext[x][1][:, None])
            bs = cst.tile([128, 1], F32, tag="bs_%s" % x)
            nc.vector.tensor_tensor(out=bs[:], in0=b1[:], in1=b2[:],
                                    op=mybir.AluOpType.add)
            bias[x] = bs

        it32 = cst.tile([128, 2 * TILE_E], mybir.dt.int32)
        nc.gpsimd.iota(out=it32[:], pattern=[[0, 2 * CPT], [1, CHUNK]], base=0,
                       channel_multiplier=0)
        iotaF = cst.tile([128, 2 * TILE_E], F32)
        nc.vector.tensor_copy(out=iotaF[:], in_=it32[:])

        dstT = cst.tile([128, T * CPT], F32)
        nc.sync.dma_start(out=dstT[:], in_=dstT_ext[:])

        # -- main loop: groups of 2 node tiles ------------------------------
        # hce cols: [h(t0) h(t1) | c(t0) c(t1) | e(t0) e(t1)], 1024 each third
        # sfdf cols: [sf(t0) sf(t1) | df(t0) df(t1)]
        # hc cols:  [h(t0) h(t1) | c(t0) c(t1)] (host unmaps)
        assert T % 2 == 0
        AF = mybir.ActivationFunctionType
        TE2 = 2 * TILE_E
        for g in range(T // 2):
            hce = pin.tile([128, 3 * TE2], F32, tag="hce")
            nc.sync.dma_start(out=hce[:], in_=hce_ext[g])
            sfdf = pin.tile([G, 2 * TE2], F32, tag="sfdf")
            nc.scalar.dma_start(out=sfdf[:], in_=sfdf_ext[g])
            B68 = pcv.tile([G + 4, TE2], BF16, tag="B68")
            nc.sync.dma_start(
                out=B68[G:G + 4, :],
                in_=oh4_ext[:, g * TE2:(g + 1) * TE2])
            nc.vector.tensor_tensor(
                out=B68[0:G, :], in0=sfdf[:, 0:TE2], in1=sfdf[:, TE2:2 * TE2],
                op=mybir.AluOpType.mult)
            M4 = pcv.tile([128, TE2], BF16, tag="M4")
            nc.vector.tensor_tensor(
                out=M4[:].rearrange("p (c j) -> p c j", c=2 * CPT),
                in0=iotaF[:].rearrange("p (c j) -> p c j", c=2 * CPT),
                in1=dstT[:, g * 2 * CPT:(g + 1) * 2 * CPT, None]
                    .to_broadcast([128, 2 * CPT, CHUNK]),
                op=mybir.AluOpType.is_equal)
            e4b = pcv.tile([128, TE2], BF16, tag="e4b")
            nc.scalar.activation(out=e4b[:], in_=hce[:, 2 * TE2:3 * TE2],
                                 func=AF.Copy)
            c4b = pcv.tile([128, TE2], BF16, tag="c4b")
            nc.gpsimd.tensor_copy(out=c4b[:], in_=hce[:, TE2:2 * TE2])

            ewb = pcv.tile([128, TE2], F32, tag="ewb")
            for tl in range(2):
                ew4 = pmm.tile([128, TILE_E], F32, tag="ew4", space="PSUM")
                for c in range(CPT):
                    lsl = slice(tl * TILE_E + c * CHUNK,
                                tl * TILE_E + (c + 1) * CHUNK)
                    nc.tensor.matmul(out=ew4[:, c * CHUNK:(c + 1) * CHUNK],
                                     lhsT=B68[:, lsl], rhs=wtcomb[:],
                                     start=True, stop=True)
                nc.scalar.activation(
                    out=ewb[:, tl * TILE_E:(tl + 1) * TILE_E], in_=ew4[:],
                    func=AF.Copy)
            hw4 = pcv.tile([128, TE2], BF16, tag="hw4")
            nc.vector.tensor_tensor(out=hw4[:], in0=hce[:, 0:TE2], in1=ewb[:],
                                    op=mybir.AluOpType.mult)

            # hs12 regions: [0:128]=t0_a [128:256]=t1_a
            #               [256:384]=t0_b [384:512]=t1_b
            hs12 = pacc.tile([128, 4 * 128], F32, tag="hs12", space="PSUM")
            cs = pacc.tile([128, 256], F32, tag="cs", space="PSUM")
            for tl in range(2):
                for lhs4, dst_ap in (
                        (hw4, hs12[:, tl * 128:(tl + 1) * 128]),
                        (e4b, hs12[:, 256 + tl * 128:256 + (tl + 1) * 128]),
                        (c4b, cs[:, tl * 128:(tl + 1) * 128])):
                    for c in range(CPT):
                        sl = slice(tl * TILE_E + c * CHUNK,
                                   tl * TILE_E + (c + 1) * CHUNK)
                        nc.tensor.matmul(out=dst_ap, lhsT=lhs4[:, sl],
                                         rhs=M4[:, sl], start=(c == 0),
                                         stop=(c == CPT - 1))

            hsab = pnd.tile([128, 512], BF16, tag="hsab")
            nc.vector.tensor_copy(out=hsab[:], in_=hs12[:])
            css = pnd.tile([128, 256], F32, tag="css")
            nc.scalar.activation(out=css[:], in_=cs[:], func=AF.Copy)

            gate = {}
            for x, fn in (("f", "Sigmoid"), ("i", "Sigmoid"),
                          ("u", "Tanh"), ("o", "Sigmoid")):
                gp = pmm.tile([128, 256], F32, tag="mm", space="PSUM")
                nc.tensor.matmul(out=gp[:], lhsT=wg[x][0][:],
                                 rhs=hsab[:, 0:256], start=True, stop=False)
                nc.tensor.matmul(out=gp[:], lhsT=wg[x][1][:],
                                 rhs=hsab[:, 256:512], start=False, stop=True)
                gs = pnd.tile([128, 256], F32, tag="g_%s" % x)
                nc.scalar.activation(out=gs[:], in_=gp[:],
                                     func=getattr(AF, fn), bias=bias[x][:])
                gate[x] = gs

            # hc cols: [h(t0) h(t1) | c(t0) c(t1)] -- all 2D ops
            hc = pnd.tile([128, 512], F32, tag="hc")
            ct = pnd.tile([128, 256], F32, tag="ct")
            nc.vector.tensor_tensor(out=ct[:], in0=gate["f"][:], in1=css[:],
                                    op=mybir.AluOpType.mult)
            iu = pnd.tile([128, 256], F32, tag="iu")
            nc.vector.tensor_tensor(out=iu[:], in0=gate["i"][:],
                                    in1=gate["u"][:], op=mybir.AluOpType.mult)
            nc.vector.tensor_tensor(out=hc[:, 256:512], in0=iu[:], in1=ct[:],
                                    op=mybir.AluOpType.add)
            th = pnd.tile([128, 256], F32, tag="th")
            nc.scalar.activation(out=th[:], in_=hc[:, 256:512], func=AF.Tanh)
            nc.vector.tensor_tensor(out=hc[:, 0:256], in0=gate["o"][:],
                                    in1=th[:], op=mybir.AluOpType.mult)
            nc.gpsimd.dma_start(
                out=out_ext[:, g * 512:(g + 1) * 512], in_=hc[:])

        for p in (pmm, pacc, pnd, pcv, pin, cst):
            p.release()
    nc.finalize()
    return nc


def plan_tiles(dst_local, npc):
    """Greedy node tiling: <=128 nodes and <=TILE_E edges per tile.
    Returns list of (n0, n1, e0, e1) using sorted-edge offsets."""
    cnt = np.bincount(dst_local, minlength=npc)
    cum = np.concatenate([[0], np.cumsum(cnt)])
    tiles = []
    s = 0
    while s < npc:
        hi = min(s + 128, npc)
        m = int(np.searchsorted(cum, cum[s] + TILE_E, side="right")) - 1
        m = max(s + 1, min(hi, m))
        tiles.append((s, m, int(cum[s]), int(cum[m])))
        s = m
    return tiles


def prep_core(k, h_src, c_src, embed_dst, src_f, dst_f, etype, dst, T=None):
    """Build one core's padded, tiled input arrays."""
    lo = k * NPC
    sel = np.nonzero((dst >= lo) & (dst < lo + NPC))[0]
    dl = (dst[sel] - lo).astype(np.int64)
    order = np.argsort(dl, kind="stable")
    eidx = sel[order]
    dls = dl[order]
    tiles = plan_tiles(dls, NPC)
    Tk = len(tiles)
    if T is None:
        T = Tk
    assert Tk <= T
    ES = T * TILE_E
    src_slot = np.full(ES, -1, dtype=np.int64)
    dst_slot = np.full(ES, -1.0, dtype=np.float32)
    for t, (n0, n1, e0, e1) in enumerate(tiles):
        ne = e1 - e0
        assert ne <= TILE_E and n1 - n0 <= 128
        src_slot[t * TILE_E:t * TILE_E + ne] = eidx[e0:e1]
        dst_slot[t * TILE_E:t * TILE_E + ne] = (dls[e0:e1] - n0).astype(np.float32)
    val = src_slot >= 0
    gi = src_slot[val]

    def pad_rows(a, w):
        out = np.zeros((ES, w), dtype=np.float32)
        out[val] = a[gi]
        return out

    def chunk_layout(a):
        # [ES, H] -> [T, 128, TILE_E] with slot (c*128+p) at [t, p, c*128:...]
        return a.reshape(T, CPT, CHUNK, H).transpose(0, 2, 1, 3) \
                .reshape(T, 128, TILE_E)

    def pair(a):
        # [T,128,W] -> [T//2,128,2W] pairing consecutive tiles along cols
        Tn, P, W = a.shape
        return a.reshape(Tn // 2, 2, P, W).transpose(0, 2, 1, 3) \
                .reshape(Tn // 2, P, 2 * W)

    hp = pair(chunk_layout(pad_rows(h_src, H)))
    cp = pair(chunk_layout(pad_rows(c_src, H)))
    ep = pair(chunk_layout(pad_rows(embed_dst, H)))
    hce = np.ascontiguousarray(np.concatenate([hp, cp, ep], axis=2))
    sf = pair(pad_rows(src_f, G).reshape(T, TILE_E, G).transpose(0, 2, 1))
    df = pair(pad_rows(dst_f, G).reshape(T, TILE_E, G).transpose(0, 2, 1))
    sfdf = np.ascontiguousarray(np.concatenate([sf, df], axis=2))
    oh = np.zeros((ES, 4), dtype=np.float32)
    oh[val, etype[gi]] = 1.0
    oh[val, 3] = 1.0
    ohT = np.ascontiguousarray(oh.T).astype(bf16_np)
    dstT = np.ascontiguousarray(dst_slot.reshape(T * CPT, CHUNK).T)
    return {"hce": hce, "sfdf": sfdf, "oh4": ohT, "dstT": dstT}, tiles, Tk


def _belB(b_el):
    out = np.zeros((4, 128), dtype=np.float32)
    out[3] = b_el
    return out


_graph_cache = {}


def kernel(**inputs):
    h_src = np.asarray(inputs["h_src"], dtype=np.float32)
    c_src = np.asarray(inputs["c_src"], dtype=np.float32)
    embed_dst = np.asarray(inputs["embed_dst"], dtype=np.float32)
    src_f = np.asarray(inputs["src_node_feat"], dtype=np.float32)
    dst_f = np.asarray(inputs["dst_node_feat"], dtype=np.float32)
    etype = np.asarray(inputs["edge_type_idx"]).astype(np.int64)
    dst = np.asarray(inputs["dst_idx"]).astype(np.int64)

    weights = {
        "welT": np.ascontiguousarray(np.asarray(inputs["W_el"], np.float32).T),
        "wa": np.ascontiguousarray(np.concatenate(
            [np.asarray(inputs["W_eoh"], np.float32),
             np.asarray(inputs["b_eoh"], np.float32)[:, None]], axis=1)),
        "belB": _belB(np.asarray(inputs["b_el"], np.float32)),
    }
    for x, wn, bwn, bn in (("f", "Wf", "bWf", "bf"), ("i", "Wi", "bWi", "bi"),
                           ("u", "Wu", "bWu", "bu"), ("o", "Wo", "bWo", "bo")):
        weights["w%sT" % x] = np.ascontiguousarray(
            np.asarray(inputs[wn], np.float32).T)
        weights["bW%s" % x] = np.asarray(inputs[bwn], np.float32)
        weights["b%s" % x] = np.asarray(inputs[bn], np.float32)

    planned = []
    for k in range(NCORES):
        lo = k * NPC
        sel = np.nonzero((dst >= lo) & (dst < lo + NPC))[0]
        dl = np.sort((dst[sel] - lo).astype(np.int64))
        planned.append(plan_tiles(dl, NPC))
    T = max(len(p) for p in planned)
    T += T % 2  # group-of-2 tiling needs even T

    in_maps = []
    tiles_all = []
    for k in range(NCORES):
        m, tiles, _ = prep_core(k, h_src, c_src, embed_dst, src_f, dst_f,
                                etype, dst, T=T)
        m.update(weights)
        in_maps.append(m)
        tiles_all.append(tiles)

    if T not in _graph_cache:
        _graph_cache[T] = build_graph(T)
    nc = _graph_cache[T]

    if TRACE:
        _install_axon_hook()
    res = run_bass_kernel_spmd(nc, in_maps, list(range(NCORES)), trace=TRACE)
    LAST["res"] = res

    out = np.empty((N, 2 * H), dtype=np.float32)
    for k in range(NCORES):
        outT = np.asarray(res.results[k]["outT"])
        for t, (n0, n1, _, _) in enumerate(tiles_all[k]):
            nn = n1 - n0
            base = k * NPC
            gbase = (t // 2) * 512 + (t % 2) * 128
            out[base + n0:base + n1, 0:H] = outT[:, gbase:gbase + nn].T
            out[base + n0:base + n1, H:2 * H] = \
                outT[:, gbase + 256:gbase + 256 + nn].T
    return out


# revision 14
# speedup vs baseline: 2.1747x; 1.6696x over previous
"""ChildSumTreeLSTMCell on 8 Trainium2 NeuronCores.

Strategy: sort edges by destination node on the host (index preprocessing as
part of sharding), partition nodes contiguously across the 8 cores so every
core's segment sums are fully local -- zero collectives.  On each core, edges
are packed into node tiles (<=128 nodes, <=512 edges each, 4 chunks of 128
edge slots).  Segment sums become matmuls against a 0/1 membership matrix M
built on-device from the destination indices; the forget-gate gather
f[dst] * c_src factorizes to f * segment_sum(c_src), removing the second
scatter entirely.  All matmuls run in bf16 (inputs stay f32 in HBM).

v2: packed DMA streams (hce = h|c|e in one 768KB transfer, sfdf merged,
single merged [h|c] output), DMA issue spread across sync/gpsimd queues,
one-shot M build via free-dim broadcast, merged PSUM evacuation.
"""

import sys

for _p in ("/opt/trn_rl_repo", "/root/.axon_site/_ro/trn_rl_repo"):
    if _p not in sys.path:
        sys.path.append(_p)

import numpy as np
import ml_dtypes

import concourse.bacc as bacc
import concourse.mybir as mybir
import concourse.tile as tile
from concourse.bass_utils import run_bass_kernel_spmd

F32 = mybir.dt.float32
BF16 = mybir.dt.bfloat16

E = 500_000
N = 125_000
H = 128
G = 64
NCORES = 8
NPC = N // NCORES          # nodes per core
CHUNK = 128                # edges per chunk
CPT = 4                    # chunks per tile
TILE_E = CHUNK * CPT       # edge slots per tile
bf16_np = ml_dtypes.bfloat16

TRACE = False              # set by test.py to capture an NTFF profile
LAST = {}                  # last run's BassKernelResults


def _install_axon_hook():
    import types, contextlib, ctypes

    def _make_hook(so_path="/opt/axon/libaxon_pjrt.so"):
        lib = ctypes.CDLL(so_path)
        if not hasattr(lib, "axon_start_nrt_profile"):
            return None
        lib.axon_start_nrt_profile.argtypes = [
            ctypes.POINTER(ctypes.c_int64), ctypes.c_size_t]
        lib.axon_start_nrt_profile.restype = ctypes.c_int64
        lib.axon_stop_nrt_profile.argtypes = [ctypes.c_char_p]
        lib.axon_stop_nrt_profile.restype = ctypes.c_int64

        @contextlib.contextmanager
        def hook(output_dir, device_ids):
            import jax
            jax.devices()
            if device_ids:
                ids = (ctypes.c_int64 * len(device_ids))(*device_ids)
                rc = lib.axon_start_nrt_profile(ids, len(device_ids))
            else:
                rc = lib.axon_start_nrt_profile(None, 0)
            if rc != 0:
                raise RuntimeError("axon_start_nrt_profile rc=%d" % rc)
            try:
                yield
            finally:
                n = lib.axon_stop_nrt_profile(str(output_dir).encode())
                print("profile: %d file(s) written to %s" % (n, output_dir),
                      file=sys.stderr)

        return hook

    hook = _make_hook()
    mod = types.ModuleType("antenv.axon_hooks")
    mod.get_axon_ntff_profile_hook = lambda: hook
    mod.set_axon_ntff_profile_hook = lambda h: None
    sys.modules["antenv.axon_hooks"] = mod


def build_graph(T):
    """Build the per-core Bass graph for T node tiles."""
    nc = bacc.Bacc()
    dp = nc.declare_dram_parameter
    hce_ext = dp("hce", [T // 2, 128, 6 * TILE_E], BF16, isOutput=False)
    sfdf_ext = dp("sfdf", [T // 2, G, 4 * TILE_E], BF16, isOutput=False)
    oh4_ext = dp("oh4", [4, T * TILE_E], BF16, isOutput=False)
    dstT_ext = dp("dstT", [128, T * CPT], BF16, isOutput=False)
    welT_ext = dp("welT", [G, 128], F32, isOutput=False)
    wa_ext = dp("wa", [G, 4], F32, isOutput=False)
    bel_ext = dp("belB", [4, 128], F32, isOutput=False)
    wg_ext = {}
    for x in "fiuo":
        wg_ext[x] = dp("w%sT" % x, [2 * H, 128], F32, isOutput=False)
    bias_ext = {}
    for x in "fiuo":
        bias_ext[x] = (dp("bW%s" % x, [H], F32, isOutput=False),
                       dp("b%s" % x, [H], F32, isOutput=False))
    out_ext = dp("outT", [128, T * 2 * H], F32, isOutput=True)

    with tile.TileContext(nc) as tc:
        cst = tc.alloc_tile_pool(name="cst", bufs=1)
        pin = tc.alloc_tile_pool(name="pin", bufs=4)
        pcv = tc.alloc_tile_pool(name="pcv", bufs=2)
        pnd = tc.alloc_tile_pool(name="pnd", bufs=2)
        pacc = tc.alloc_tile_pool(name="pacc", bufs=2, space="PSUM")
        pmm = tc.alloc_tile_pool(name="pmm", bufs=2, space="PSUM")

        # -- setup: constants -----------------------------------------------
        welT_sb = cst.tile([G, 128], F32)
        nc.sync.dma_start(out=welT_sb[:], in_=welT_ext[:])
        wa_sb = cst.tile([G, 4], F32)
        nc.sync.dma_start(out=wa_sb[:], in_=wa_ext[:])
        belr = cst.tile([4, 128], F32)
        nc.sync.dma_start(out=belr[:], in_=bel_ext[:])
        t2p = pmm.tile([4, 128], F32, tag="mm")
        nc.tensor.matmul(out=t2p[:], lhsT=wa_sb[:], rhs=welT_sb[:],
                         start=True, stop=True)
        wel_b16 = cst.tile([G, 128], BF16)
        nc.vector.tensor_copy(out=wel_b16[:], in_=welT_sb[:])
        t4b = cst.tile([4, 128], BF16)
        nc.vector.tensor_tensor(out=t4b[:], in0=t2p[:],
                                in1=belr[:], op=mybir.AluOpType.add)
        wtcomb = cst.tile([G + 4, 128], BF16)
        nc.sync.dma_start(out=wtcomb[0:G, :], in_=wel_b16[:])
        nc.sync.dma_start(out=wtcomb[G:G + 4, :], in_=t4b[:])

        wg = {}
        for x in "fiuo":
            stg = cst.tile([128, 128], F32, tag="wstg_%s" % x)
            nc.sync.dma_start(out=stg[:], in_=wg_ext[x][0:128, :])
            wa_t = cst.tile([128, 128], BF16, tag="wg_%s_a" % x)
            nc.vector.tensor_copy(out=wa_t[:], in_=stg[:])
            stg2 = cst.tile([128, 128], F32, tag="wstg2_%s" % x)
            nc.sync.dma_start(out=stg2[:], in_=wg_ext[x][128:256, :])
            wb_t = cst.tile([128, 128], BF16, tag="wg_%s_b" % x)
            nc.vector.tensor_copy(out=wb_t[:], in_=stg2[:])
            wg[x] = (wa_t, wb_t)

        bias = {}
        for x in "fiuo":
            b1 = cst.tile([128, 1], F32, tag="b1_%s" % x)
            nc.sync.dma_start(out=b1[:], in_=bias_ext[x][0][:, None])
            b2 = cst.tile([128, 1], F32, tag="b2_%s" % x)
            nc.sync.dma_start(out=b2[:], in_=bias_ext[x][1][:, None])
            bs = cst.tile([128, 1], F32, tag="bs_%s" % x)
            nc.vector.tensor_tensor(out=bs[:], in0=b1[:], in1=b2[:],
                                    op=mybir.AluOpType.add)
            bias[x] = bs

        it32 = cst.tile([128, 2 * TILE_E], mybir.dt.int32)
        nc.gpsimd.iota(out=it32[:], pattern=[[0, 2 * CPT], [1, CHUNK]], base=0,
                       channel_multiplier=0)
        iotaF = cst.tile([128, 2 * TILE_E], BF16)
        nc.vector.tensor_copy(out=iotaF[:], in_=it32[:])

        dstT = cst.tile([128, T * CPT], BF16)
        nc.sync.dma_start(out=dstT[:], in_=dstT_ext[:])

        # -- main loop: groups of 2 node tiles ------------------------------
        # hce cols (bf16): [h(t0) h(t1) | c(t0) c(t1) | e(t0) e(t1)]
        # sfdf cols (bf16): [sf(t0) sf(t1) | df(t0) df(t1)]
        # hc cols (f32):  [h(t0) h(t1) | c(t0) c(t1)] (host unmaps)
        assert T % 2 == 0
        AF = mybir.ActivationFunctionType
        TE2 = 2 * TILE_E
        for g in range(T // 2):
            hce = pin.tile([128, 3 * TE2], BF16, tag="hce")
            nc.sync.dma_start(out=hce[:], in_=hce_ext[g])
            sfdf = pin.tile([G, 2 * TE2], BF16, tag="sfdf")
            nc.scalar.dma_start(out=sfdf[:], in_=sfdf_ext[g])
            B68 = pcv.tile([G + 4, TE2], BF16, tag="B68")
            nc.sync.dma_start(
                out=B68[G:G + 4, :],
                in_=oh4_ext[:, g * TE2:(g + 1) * TE2])
            nc.vector.tensor_tensor(
                out=B68[0:G, :], in0=sfdf[:, 0:TE2], in1=sfdf[:, TE2:2 * TE2],
                op=mybir.AluOpType.mult)
            M4 = pcv.tile([128, TE2], BF16, tag="M4")
            nc.vector.tensor_tensor(
                out=M4[:].rearrange("p (c j) -> p c j", c=2 * CPT),
                in0=iotaF[:].rearrange("p (c j) -> p c j", c=2 * CPT),
                in1=dstT[:, g * 2 * CPT:(g + 1) * 2 * CPT, None]
                    .to_broadcast([128, 2 * CPT, CHUNK]),
                op=mybir.AluOpType.is_equal)

            ewb = pcv.tile([128, TE2], BF16, tag="ewb")
            for tl in range(2):
                ew4 = pmm.tile([128, TILE_E], F32, tag="ew4", space="PSUM")
                for c in range(CPT):
                    lsl = slice(tl * TILE_E + c * CHUNK,
                                tl * TILE_E + (c + 1) * CHUNK)
                    nc.tensor.matmul(out=ew4[:, c * CHUNK:(c + 1) * CHUNK],
                                     lhsT=B68[:, lsl], rhs=wtcomb[:],
                                     start=True, stop=True)
                nc.scalar.activation(
                    out=ewb[:, tl * TILE_E:(tl + 1) * TILE_E], in_=ew4[:],
                    func=AF.Copy)
            hw4 = pcv.tile([128, TE2], BF16, tag="hw4")
            nc.vector.tensor_tensor(out=hw4[:], in0=hce[:, 0:TE2], in1=ewb[:],
                                    op=mybir.AluOpType.mult)

            # hs12 regions: [0:128]=t0_a [128:256]=t1_a
            #               [256:384]=t0_b [384:512]=t1_b
            hs12 = pacc.tile([128, 4 * 128], F32, tag="hs12", space="PSUM")
            cs = pacc.tile([128, 256], F32, tag="cs", space="PSUM")
            for tl in range(2):
                for lhs4, off, dst_ap in (
                        (hw4, 0, hs12[:, tl * 128:(tl + 1) * 128]),
                        (hce, 2 * TE2,
                         hs12[:, 256 + tl * 128:256 + (tl + 1) * 128]),
                        (hce, TE2, cs[:, tl * 128:(tl + 1) * 128])):
                    for c in range(CPT):
                        lo = tl * TILE_E + c * CHUNK
                        sl = slice(lo, lo + CHUNK)
                        nc.tensor.matmul(
                            out=dst_ap, lhsT=lhs4[:, off + lo:off + lo + CHUNK],
                            rhs=M4[:, sl], start=(c == 0),
                            stop=(c == CPT - 1))

            hsab = pnd.tile([128, 512], BF16, tag="hsab")
            nc.vector.tensor_copy(out=hsab[:], in_=hs12[:])
            css = pnd.tile([128, 256], F32, tag="css")
            nc.scalar.activation(out=css[:], in_=cs[:], func=AF.Copy)

            gate = {}
            for x, fn in (("f", "Sigmoid"), ("i", "Sigmoid"),
                          ("u", "Tanh"), ("o", "Sigmoid")):
                gp = pmm.tile([128, 256], F32, tag="mm", space="PSUM")
                nc.tensor.matmul(out=gp[:], lhsT=wg[x][0][:],
                                 rhs=hsab[:, 0:256], start=True, stop=False)
                nc.tensor.matmul(out=gp[:], lhsT=wg[x][1][:],
                                 rhs=hsab[:, 256:512], start=False, stop=True)
                gs = pnd.tile([128, 256], F32, tag="g_%s" % x)
                nc.scalar.activation(out=gs[:], in_=gp[:],
                                     func=getattr(AF, fn), bias=bias[x][:])
                gate[x] = gs

            # hc cols: [h(t0) h(t1) | c(t0) c(t1)] -- all 2D ops
            hc = pnd.tile([128, 512], F32, tag="hc")
            ct = pnd.tile([128, 256], F32, tag="ct")
            nc.vector.tensor_tensor(out=ct[:], in0=gate["f"][:], in1=css[:],
                                    op=mybir.AluOpType.mult)
            iu = pnd.tile([128, 256], F32, tag="iu")
            nc.gpsimd.tensor_tensor(out=iu[:], in0=gate["i"][:],
                                    in1=gate["u"][:], op=mybir.AluOpType.mult)
            nc.vector.tensor_tensor(out=hc[:, 256:512], in0=iu[:], in1=ct[:],
                                    op=mybir.AluOpType.add)
            th = pnd.tile([128, 256], F32, tag="th")
            nc.scalar.activation(out=th[:], in_=hc[:, 256:512], func=AF.Tanh)
            nc.gpsimd.tensor_tensor(out=hc[:, 0:256], in0=gate["o"][:],
                                    in1=th[:], op=mybir.AluOpType.mult)
            nc.gpsimd.dma_start(
                out=out_ext[:, g * 512:(g + 1) * 512], in_=hc[:])

        for p in (pmm, pacc, pnd, pcv, pin, cst):
            p.release()
    nc.finalize()
    return nc


def plan_tiles(dst_local, npc):
    """Greedy node tiling: <=128 nodes and <=TILE_E edges per tile.
    Returns list of (n0, n1, e0, e1) using sorted-edge offsets."""
    cnt = np.bincount(dst_local, minlength=npc)
    cum = np.concatenate([[0], np.cumsum(cnt)])
    tiles = []
    s = 0
    while s < npc:
        hi = min(s + 128, npc)
        m = int(np.searchsorted(cum, cum[s] + TILE_E, side="right")) - 1
        m = max(s + 1, min(hi, m))
        tiles.append((s, m, int(cum[s]), int(cum[m])))
        s = m
    return tiles


def prep_core(k, h_src, c_src, embed_dst, src_f, dst_f, etype, dst, T=None):
    """Build one core's padded, tiled input arrays."""
    lo = k * NPC
    sel = np.nonzero((dst >= lo) & (dst < lo + NPC))[0]
    dl = (dst[sel] - lo).astype(np.int64)
    order = np.argsort(dl, kind="stable")
    eidx = sel[order]
    dls = dl[order]
    tiles = plan_tiles(dls, NPC)
    Tk = len(tiles)
    if T is None:
        T = Tk
    assert Tk <= T
    ES = T * TILE_E
    src_slot = np.full(ES, -1, dtype=np.int64)
    dst_slot = np.full(ES, -1.0, dtype=np.float32)
    for t, (n0, n1, e0, e1) in enumerate(tiles):
        ne = e1 - e0
        assert ne <= TILE_E and n1 - n0 <= 128
        src_slot[t * TILE_E:t * TILE_E + ne] = eidx[e0:e1]
        dst_slot[t * TILE_E:t * TILE_E + ne] = (dls[e0:e1] - n0).astype(np.float32)
    val = src_slot >= 0
    gi = src_slot[val]

    def pad_rows(a, w):
        out = np.zeros((ES, w), dtype=np.float32)
        out[val] = a[gi]
        return out

    def chunk_layout(a):
        # [ES, H] -> [T, 128, TILE_E] with slot (c*128+p) at [t, p, c*128:...]
        return a.reshape(T, CPT, CHUNK, H).transpose(0, 2, 1, 3) \
                .reshape(T, 128, TILE_E)

    def pair(a):
        # [T,128,W] -> [T//2,128,2W] pairing consecutive tiles along cols
        Tn, P, W = a.shape
        return a.reshape(Tn // 2, 2, P, W).transpose(0, 2, 1, 3) \
                .reshape(Tn // 2, P, 2 * W)

    hp = pair(chunk_layout(pad_rows(h_src, H)))
    cp = pair(chunk_layout(pad_rows(c_src, H)))
    ep = pair(chunk_layout(pad_rows(embed_dst, H)))
    hce = np.ascontiguousarray(
        np.concatenate([hp, cp, ep], axis=2)).astype(bf16_np)
    sf = pair(pad_rows(src_f, G).reshape(T, TILE_E, G).transpose(0, 2, 1))
    df = pair(pad_rows(dst_f, G).reshape(T, TILE_E, G).transpose(0, 2, 1))
    sfdf = np.ascontiguousarray(
        np.concatenate([sf, df], axis=2)).astype(bf16_np)
    oh = np.zeros((ES, 4), dtype=np.float32)
    oh[val, etype[gi]] = 1.0
    oh[val, 3] = 1.0
    ohT = np.ascontiguousarray(oh.T).astype(bf16_np)
    dstT = np.ascontiguousarray(
        dst_slot.reshape(T * CPT, CHUNK).T).astype(bf16_np)
    return {"hce": hce, "sfdf": sfdf, "oh4": ohT, "dstT": dstT}, tiles, Tk


def _belB(b_el):
    out = np.zeros((4, 128), dtype=np.float32)
    out[3] = b_el
    return out


_graph_cache = {}


def kernel(**inputs):
    h_src = np.asarray(inputs["h_src"], dtype=np.float32)
    c_src = np.asarray(inputs["c_src"], dtype=np.float32)
    embed_dst = np.asarray(inputs["embed_dst"], dtype=np.float32)
    src_f = np.asarray(inputs["src_node_feat"], dtype=np.float32)
    dst_f = np.asarray(inputs["dst_node_feat"], dtype=np.float32)
    etype = np.asarray(inputs["edge_type_idx"]).astype(np.int64)
    dst = np.asarray(inputs["dst_idx"]).astype(np.int64)

    weights = {
        "welT": np.ascontiguousarray(np.asarray(inputs["W_el"], np.float32).T),
        "wa": np.ascontiguousarray(np.concatenate(
            [np.asarray(inputs["W_eoh"], np.float32),
             np.asarray(inputs["b_eoh"], np.float32)[:, None]], axis=1)),
        "belB": _belB(np.asarray(inputs["b_el"], np.float32)),
    }
    for x, wn, bwn, bn in (("f", "Wf", "bWf", "bf"), ("i", "Wi", "bWi", "bi"),
                           ("u", "Wu", "bWu", "bu"), ("o", "Wo", "bWo", "bo")):
        weights["w%sT" % x] = np.ascontiguousarray(
            np.asarray(inputs[wn], np.float32).T)
        weights["bW%s" % x] = np.asarray(inputs[bwn], np.float32)
        weights["b%s" % x] = np.asarray(inputs[bn], np.float32)

    planned = []
    for k in range(NCORES):
        lo = k * NPC
        sel = np.nonzero((dst >= lo) & (dst < lo + NPC))[0]
        dl = np.sort((dst[sel] - lo).astype(np.int64))
        planned.append(plan_tiles(dl, NPC))
    T = max(len(p) for p in planned)
    T += T % 2  # group-of-2 tiling needs even T

    in_maps = []
    tiles_all = []
    for k in range(NCORES):
        m, tiles, _ = prep_core(k, h_src, c_src, embed_dst, src_f, dst_f,
                                etype, dst, T=T)
        m.update(weights)
        in_maps.append(m)
        tiles_all.append(tiles)

    if T not in _graph_cache:
        _graph_cache[T] = build_graph(T)
    nc = _graph_cache[T]

    if TRACE:
        _install_axon_hook()
    res = run_bass_kernel_spmd(nc, in_maps, list(range(NCORES)), trace=TRACE)
    LAST["res"] = res

    out = np.empty((N, 2 * H), dtype=np.float32)
    for k in range(NCORES):
        outT = np.asarray(res.results[k]["outT"])
        for t, (n0, n1, _, _) in enumerate(tiles_all[k]):
            nn = n1 - n0
            base = k * NPC
            gbase = (t // 2) * 512 + (t % 2) * 128
            out[base + n0:base + n1, 0:H] = outT[:, gbase:gbase + nn].T
            out[base + n0:base + n1, H:2 * H] = \
                outT[:, gbase + 256:gbase + 256 + nn].T
    return out


# revision 16
# speedup vs baseline: 2.2173x; 1.0196x over previous
"""ChildSumTreeLSTMCell on 8 Trainium2 NeuronCores.

Strategy: sort edges by destination node on the host (index preprocessing as
part of sharding), partition nodes contiguously across the 8 cores so every
core's segment sums are fully local -- zero collectives.  On each core, edges
are packed into node tiles (<=128 nodes, <=512 edges each, 4 chunks of 128
edge slots).  Segment sums become matmuls against a 0/1 membership matrix M
built on-device from the destination indices; the forget-gate gather
f[dst] * c_src factorizes to f * segment_sum(c_src), removing the second
scatter entirely.  All matmuls run in bf16 (inputs stay f32 in HBM).

v2: packed DMA streams (hce = h|c|e in one 768KB transfer, sfdf merged,
single merged [h|c] output), DMA issue spread across sync/gpsimd queues,
one-shot M build via free-dim broadcast, merged PSUM evacuation.
"""

import sys

for _p in ("/opt/trn_rl_repo", "/root/.axon_site/_ro/trn_rl_repo"):
    if _p not in sys.path:
        sys.path.append(_p)

import numpy as np
import ml_dtypes

import concourse.bacc as bacc
import concourse.mybir as mybir
import concourse.tile as tile
from concourse.bass_utils import run_bass_kernel_spmd

F32 = mybir.dt.float32
BF16 = mybir.dt.bfloat16

E = 500_000
N = 125_000
H = 128
G = 64
NCORES = 8
NPC = N // NCORES          # nodes per core
CHUNK = 128                # edges per chunk
CPT = 4                    # chunks per tile
TILE_E = CHUNK * CPT       # edge slots per tile
bf16_np = ml_dtypes.bfloat16

TRACE = False              # set by test.py to capture an NTFF profile
LAST = {}                  # last run's BassKernelResults


def _install_axon_hook():
    import types, contextlib, ctypes

    def _make_hook(so_path="/opt/axon/libaxon_pjrt.so"):
        lib = ctypes.CDLL(so_path)
        if not hasattr(lib, "axon_start_nrt_profile"):
            return None
        lib.axon_start_nrt_profile.argtypes = [
            ctypes.POINTER(ctypes.c_int64), ctypes.c_size_t]
        lib.axon_start_nrt_profile.restype = ctypes.c_int64
        lib.axon_stop_nrt_profile.argtypes = [ctypes.c_char_p]
        lib.axon_stop_nrt_profile.restype = ctypes.c_int64

        @contextlib.contextmanager
        def hook(output_dir, device_ids):
            import jax
            jax.devices()
            if device_ids:
                ids = (ctypes.c_int64 * len(device_ids))(*device_ids)
                rc = lib.axon_start_nrt_profile(ids, len(device_ids))
            else:
                rc = lib.axon_start_nrt_profile(None, 0)
            if rc != 0:
                raise RuntimeError("axon_start_nrt_profile rc=%d" % rc)
            try:
                yield
            finally:
                n = lib.axon_stop_nrt_profile(str(output_dir).encode())
                print("profile: %d file(s) written to %s" % (n, output_dir),
                      file=sys.stderr)

        return hook

    hook = _make_hook()
    mod = types.ModuleType("antenv.axon_hooks")
    mod.get_axon_ntff_profile_hook = lambda: hook
    mod.set_axon_ntff_profile_hook = lambda h: None
    sys.modules["antenv.axon_hooks"] = mod


def build_graph(T):
    """Build the per-core Bass graph for T node tiles."""
    nc = bacc.Bacc()
    dp = nc.declare_dram_parameter
    hce_ext = dp("hce", [T // 2, 128, 6 * TILE_E], BF16, isOutput=False)
    sfdf_ext = dp("sfdf", [T // 2, G, 4 * TILE_E], BF16, isOutput=False)
    oh4_ext = dp("oh4", [4, T * TILE_E], BF16, isOutput=False)
    dstT_ext = dp("dstT", [128, T * CPT], BF16, isOutput=False)
    welT_ext = dp("welT", [G, 128], F32, isOutput=False)
    wa_ext = dp("wa", [G, 4], F32, isOutput=False)
    bel_ext = dp("belB", [4, 128], F32, isOutput=False)
    wg_ext = {}
    for x in "fiuo":
        wg_ext[x] = dp("w%sT" % x, [2 * H, 128], F32, isOutput=False)
    bias_ext = {}
    for x in "fiuo":
        bias_ext[x] = (dp("bW%s" % x, [H], F32, isOutput=False),
                       dp("b%s" % x, [H], F32, isOutput=False))
    out_ext = dp("outT", [128, T * 2 * H], F32, isOutput=True)

    with tile.TileContext(nc) as tc:
        cst = tc.alloc_tile_pool(name="cst", bufs=1)
        pin = tc.alloc_tile_pool(name="pin", bufs=6)
        pcv = tc.alloc_tile_pool(name="pcv", bufs=3)
        pnd = tc.alloc_tile_pool(name="pnd", bufs=3)
        pacc = tc.alloc_tile_pool(name="pacc", bufs=2, space="PSUM")
        pmm = tc.alloc_tile_pool(name="pmm", bufs=2, space="PSUM")

        # -- setup: constants -----------------------------------------------
        welT_sb = cst.tile([G, 128], F32)
        nc.sync.dma_start(out=welT_sb[:], in_=welT_ext[:])
        wa_sb = cst.tile([G, 4], F32)
        nc.sync.dma_start(out=wa_sb[:], in_=wa_ext[:])
        belr = cst.tile([4, 128], F32)
        nc.sync.dma_start(out=belr[:], in_=bel_ext[:])
        t2p = pmm.tile([4, 128], F32, tag="mm")
        nc.tensor.matmul(out=t2p[:], lhsT=wa_sb[:], rhs=welT_sb[:],
                         start=True, stop=True)
        wel_b16 = cst.tile([G, 128], BF16)
        nc.vector.tensor_copy(out=wel_b16[:], in_=welT_sb[:])
        t4b = cst.tile([4, 128], BF16)
        nc.vector.tensor_tensor(out=t4b[:], in0=t2p[:],
                                in1=belr[:], op=mybir.AluOpType.add)
        wtcomb = cst.tile([G + 4, 128], BF16)
        nc.sync.dma_start(out=wtcomb[0:G, :], in_=wel_b16[:])
        nc.sync.dma_start(out=wtcomb[G:G + 4, :], in_=t4b[:])

        wg = {}
        for x in "fiuo":
            stg = cst.tile([128, 128], F32, tag="wstg_%s" % x)
            nc.sync.dma_start(out=stg[:], in_=wg_ext[x][0:128, :])
            wa_t = cst.tile([128, 128], BF16, tag="wg_%s_a" % x)
            nc.vector.tensor_copy(out=wa_t[:], in_=stg[:])
            stg2 = cst.tile([128, 128], F32, tag="wstg2_%s" % x)
            nc.sync.dma_start(out=stg2[:], in_=wg_ext[x][128:256, :])
            wb_t = cst.tile([128, 128], BF16, tag="wg_%s_b" % x)
            nc.vector.tensor_copy(out=wb_t[:], in_=stg2[:])
            wg[x] = (wa_t, wb_t)

        bias = {}
        for x in "fiuo":
            b1 = cst.tile([128, 1], F32, tag="b1_%s" % x)
            nc.sync.dma_start(out=b1[:], in_=bias_ext[x][0][:, None])
            b2 = cst.tile([128, 1], F32, tag="b2_%s" % x)
            nc.sync.dma_start(out=b2[:], in_=bias_ext[x][1][:, None])
            bs = cst.tile([128, 1], F32, tag="bs_%s" % x)
            nc.vector.tensor_tensor(out=bs[:], in0=b1[:], in1=b2[:],
                                    op=mybir.AluOpType.add)
            bias[x] = bs

        it32 = cst.tile([128, 2 * TILE_E], mybir.dt.int32)
        nc.gpsimd.iota(out=it32[:], pattern=[[0, 2 * CPT], [1, CHUNK]], base=0,
                       channel_multiplier=0)
        iotaF = cst.tile([128, 2 * TILE_E], BF16)
        nc.vector.tensor_copy(out=iotaF[:], in_=it32[:])

        dstT = cst.tile([128, T * CPT], BF16)
        nc.sync.dma_start(out=dstT[:], in_=dstT_ext[:])

        # -- main loop: groups of 2 node tiles ------------------------------
        # hce cols (bf16): [h(t0) h(t1) | c(t0) c(t1) | e(t0) e(t1)]
        # sfdf cols (bf16): [sf(t0) sf(t1) | df(t0) df(t1)]
        # hc cols (f32):  [h(t0) h(t1) | c(t0) c(t1)] (host unmaps)
        assert T % 2 == 0
        AF = mybir.ActivationFunctionType
        TE2 = 2 * TILE_E
        for g in range(T // 2):
            hce = pin.tile([128, 3 * TE2], BF16, tag="hce")
            nc.sync.dma_start(out=hce[:], in_=hce_ext[g])
            sfdf = pin.tile([G, 2 * TE2], BF16, tag="sfdf")
            nc.scalar.dma_start(out=sfdf[:], in_=sfdf_ext[g])
            B68 = pcv.tile([G + 4, TE2], BF16, tag="B68")
            nc.sync.dma_start(
                out=B68[G:G + 4, :],
                in_=oh4_ext[:, g * TE2:(g + 1) * TE2])
            nc.vector.tensor_tensor(
                out=B68[0:G, :], in0=sfdf[:, 0:TE2], in1=sfdf[:, TE2:2 * TE2],
                op=mybir.AluOpType.mult)
            M4 = pcv.tile([128, TE2], BF16, tag="M4")
            nc.vector.tensor_tensor(
                out=M4[:].rearrange("p (c j) -> p c j", c=2 * CPT),
                in0=iotaF[:].rearrange("p (c j) -> p c j", c=2 * CPT),
                in1=dstT[:, g * 2 * CPT:(g + 1) * 2 * CPT, None]
                    .to_broadcast([128, 2 * CPT, CHUNK]),
                op=mybir.AluOpType.is_equal)

            ewb = pcv.tile([128, TE2], BF16, tag="ewb")
            for tl in range(2):
                ew4 = pmm.tile([128, TILE_E], F32, tag="ew4", space="PSUM")
                for c in range(CPT):
                    lsl = slice(tl * TILE_E + c * CHUNK,
                                tl * TILE_E + (c + 1) * CHUNK)
                    nc.tensor.matmul(out=ew4[:, c * CHUNK:(c + 1) * CHUNK],
                                     lhsT=B68[:, lsl], rhs=wtcomb[:],
                                     start=True, stop=True)
                if tl == 0:
                    nc.vector.tensor_copy(
                        out=ewb[:, tl * TILE_E:(tl + 1) * TILE_E], in_=ew4[:])
                else:
                    nc.scalar.activation(
                        out=ewb[:, tl * TILE_E:(tl + 1) * TILE_E], in_=ew4[:],
                        func=AF.Copy)
            hw4 = pcv.tile([128, TE2], BF16, tag="hw4")
            nc.vector.tensor_tensor(out=hw4[:], in0=hce[:, 0:TE2], in1=ewb[:],
                                    op=mybir.AluOpType.mult)

            # hs12 regions: [0:128]=t0_a [128:256]=t1_a
            #               [256:384]=t0_b [384:512]=t1_b
            hs12 = pacc.tile([128, 4 * 128], F32, tag="hs12", space="PSUM")
            cs = pacc.tile([128, 256], F32, tag="cs", space="PSUM")
            for tl in range(2):
                for lhs4, off, dst_ap in (
                        (hw4, 0, hs12[:, tl * 128:(tl + 1) * 128]),
                        (hce, 2 * TE2,
                         hs12[:, 256 + tl * 128:256 + (tl + 1) * 128]),
                        (hce, TE2, cs[:, tl * 128:(tl + 1) * 128])):
                    for c in range(CPT):
                        lo = tl * TILE_E + c * CHUNK
                        sl = slice(lo, lo + CHUNK)
                        nc.tensor.matmul(
                            out=dst_ap, lhsT=lhs4[:, off + lo:off + lo + CHUNK],
                            rhs=M4[:, sl], start=(c == 0),
                            stop=(c == CPT - 1))

            hsab = pnd.tile([128, 512], BF16, tag="hsab")
            nc.vector.tensor_copy(out=hsab[:], in_=hs12[:])
            css = pnd.tile([128, 256], F32, tag="css")
            nc.scalar.activation(out=css[:], in_=cs[:], func=AF.Copy)

            gate = {}
            for x, fn in (("f", "Sigmoid"), ("i", "Sigmoid"),
                          ("u", "Tanh"), ("o", "Sigmoid")):
                gp = pmm.tile([128, 256], F32, tag="mm", space="PSUM")
                nc.tensor.matmul(out=gp[:], lhsT=wg[x][0][:],
                                 rhs=hsab[:, 0:256], start=True, stop=False)
                nc.tensor.matmul(out=gp[:], lhsT=wg[x][1][:],
                                 rhs=hsab[:, 256:512], start=False, stop=True)
                gs = pnd.tile([128, 256], F32, tag="g_%s" % x)
                nc.scalar.activation(out=gs[:], in_=gp[:],
                                     func=getattr(AF, fn), bias=bias[x][:])
                gate[x] = gs

            # hc cols: [h(t0) h(t1) | c(t0) c(t1)] -- all 2D ops
            hc = pnd.tile([128, 512], F32, tag="hc")
            ct = pnd.tile([128, 256], F32, tag="ct")
            nc.vector.tensor_tensor(out=ct[:], in0=gate["f"][:], in1=css[:],
                                    op=mybir.AluOpType.mult)
            iu = pnd.tile([128, 256], F32, tag="iu")
            nc.gpsimd.tensor_tensor(out=iu[:], in0=gate["i"][:],
                                    in1=gate["u"][:], op=mybir.AluOpType.mult)
            nc.vector.tensor_tensor(out=hc[:, 256:512], in0=iu[:], in1=ct[:],
                                    op=mybir.AluOpType.add)
            th = pnd.tile([128, 256], F32, tag="th")
            nc.scalar.activation(out=th[:], in_=hc[:, 256:512], func=AF.Tanh)
            nc.gpsimd.tensor_tensor(out=hc[:, 0:256], in0=gate["o"][:],
                                    in1=th[:], op=mybir.AluOpType.mult)
            nc.sync.dma_start(
                out=out_ext[:, g * 512:(g + 1) * 512], in_=hc[:])

        for p in (pmm, pacc, pnd, pcv, pin, cst):
            p.release()
    nc.finalize()
    return nc


def plan_tiles(dst_local, npc):
    """Greedy node tiling: <=128 nodes and <=TILE_E edges per tile.
    Returns list of (n0, n1, e0, e1) using sorted-edge offsets."""
    cnt = np.bincount(dst_local, minlength=npc)
    cum = np.concatenate([[0], np.cumsum(cnt)])
    tiles = []
    s = 0
    while s < npc:
        hi = min(s + 128, npc)
        m = int(np.searchsorted(cum, cum[s] + TILE_E, side="right")) - 1
        m = max(s + 1, min(hi, m))
        tiles.append((s, m, int(cum[s]), int(cum[m])))
        s = m
    return tiles


def prep_core(k, h_src, c_src, embed_dst, src_f, dst_f, etype, dst, T=None):
    """Build one core's padded, tiled input arrays."""
    lo = k * NPC
    sel = np.nonzero((dst >= lo) & (dst < lo + NPC))[0]
    dl = (dst[sel] - lo).astype(np.int64)
    order = np.argsort(dl, kind="stable")
    eidx = sel[order]
    dls = dl[order]
    tiles = plan_tiles(dls, NPC)
    Tk = len(tiles)
    if T is None:
        T = Tk
    assert Tk <= T
    ES = T * TILE_E
    src_slot = np.full(ES, -1, dtype=np.int64)
    dst_slot = np.full(ES, -1.0, dtype=np.float32)
    for t, (n0, n1, e0, e1) in enumerate(tiles):
        ne = e1 - e0
        assert ne <= TILE_E and n1 - n0 <= 128
        src_slot[t * TILE_E:t * TILE_E + ne] = eidx[e0:e1]
        dst_slot[t * TILE_E:t * TILE_E + ne] = (dls[e0:e1] - n0).astype(np.float32)
    val = src_slot >= 0
    gi = src_slot[val]

    def pad_rows(a, w):
        out = np.zeros((ES, w), dtype=np.float32)
        out[val] = a[gi]
        return out

    def chunk_layout(a):
        # [ES, H] -> [T, 128, TILE_E] with slot (c*128+p) at [t, p, c*128:...]
        return a.reshape(T, CPT, CHUNK, H).transpose(0, 2, 1, 3) \
                .reshape(T, 128, TILE_E)

    def pair(a):
        # [T,128,W] -> [T//2,128,2W] pairing consecutive tiles along cols
        Tn, P, W = a.shape
        return a.reshape(Tn // 2, 2, P, W).transpose(0, 2, 1, 3) \
                .reshape(Tn // 2, P, 2 * W)

    hp = pair(chunk_layout(pad_rows(h_src, H)))
    cp = pair(chunk_layout(pad_rows(c_src, H)))
    ep = pair(chunk_layout(pad_rows(embed_dst, H)))
    hce = np.ascontiguousarray(
        np.concatenate([hp, cp, ep], axis=2)).astype(bf16_np)
    sf = pair(pad_rows(src_f, G).reshape(T, TILE_E, G).transpose(0, 2, 1))
    df = pair(pad_rows(dst_f, G).reshape(T, TILE_E, G).transpose(0, 2, 1))
    sfdf = np.ascontiguousarray(
        np.concatenate([sf, df], axis=2)).astype(bf16_np)
    oh = np.zeros((ES, 4), dtype=np.float32)
    oh[val, etype[gi]] = 1.0
    oh[val, 3] = 1.0
    ohT = np.ascontiguousarray(oh.T).astype(bf16_np)
    dstT = np.ascontiguousarray(
        dst_slot.reshape(T * CPT, CHUNK).T).astype(bf16_np)
    return {"hce": hce, "sfdf": sfdf, "oh4": ohT, "dstT": dstT}, tiles, Tk


def _belB(b_el):
    out = np.zeros((4, 128), dtype=np.float32)
    out[3] = b_el
    return out


_graph_cache = {}


def kernel(**inputs):
    h_src = np.asarray(inputs["h_src"], dtype=np.float32)
    c_src = np.asarray(inputs["c_src"], dtype=np.float32)
    embed_dst = np.asarray(inputs["embed_dst"], dtype=np.float32)
    src_f = np.asarray(inputs["src_node_feat"], dtype=np.float32)
    dst_f = np.asarray(inputs["dst_node_feat"], dtype=np.float32)
    etype = np.asarray(inputs["edge_type_idx"]).astype(np.int64)
    dst = np.asarray(inputs["dst_idx"]).astype(np.int64)

    weights = {
        "welT": np.ascontiguousarray(np.asarray(inputs["W_el"], np.float32).T),
        "wa": np.ascontiguousarray(np.concatenate(
            [np.asarray(inputs["W_eoh"], np.float32),
             np.asarray(inputs["b_eoh"], np.float32)[:, None]], axis=1)),
        "belB": _belB(np.asarray(inputs["b_el"], np.float32)),
    }
    for x, wn, bwn, bn in (("f", "Wf", "bWf", "bf"), ("i", "Wi", "bWi", "bi"),
                           ("u", "Wu", "bWu", "bu"), ("o", "Wo", "bWo", "bo")):
        weights["w%sT" % x] = np.ascontiguousarray(
            np.asarray(inputs[wn], np.float32).T)
        weights["bW%s" % x] = np.asarray(inputs[bwn], np.float32)
        weights["b%s" % x] = np.asarray(inputs[bn], np.float32)

    planned = []
    for k in range(NCORES):
        lo = k * NPC
        sel = np.nonzero((dst >= lo) & (dst < lo + NPC))[0]
        dl = np.sort((dst[sel] - lo).astype(np.int64))
        planned.append(plan_tiles(dl, NPC))
    T = max(len(p) for p in planned)
    T += T % 2  # group-of-2 tiling needs even T

    in_maps = []
    tiles_all = []
    for k in range(NCORES):
        m, tiles, _ = prep_core(k, h_src, c_src, embed_dst, src_f, dst_f,
                                etype, dst, T=T)
        m.update(weights)
        in_maps.append(m)
        tiles_all.append(tiles)

    if T not in _graph_cache:
        _graph_cache[T] = build_graph(T)
    nc = _graph_cache[T]

    if TRACE:
        _install_axon_hook()
    res = run_bass_kernel_spmd(nc, in_maps, list(range(NCORES)), trace=TRACE)
    LAST["res"] = res

    out = np.empty((N, 2 * H), dtype=np.float32)
    for k in range(NCORES):
        outT = np.asarray(res.results[k]["outT"])
        for t, (n0, n1, _, _) in enumerate(tiles_all[k]):
            nn = n1 - n0
            base = k * NPC
            gbase = (t // 2) * 512 + (t % 2) * 128
            out[base + n0:base + n1, 0:H] = outT[:, gbase:gbase + nn].T
            out[base + n0:base + n1, H:2 * H] = \
                outT[:, gbase + 256:gbase + 256 + nn].T
    return out
